# revision 1
# baseline (speedup 1.0000x reference)
"""Trainium2 Bass kernel for nn_LinearQuantizerModel.

MLP 1024->894->763->501 (leaky_relu 0.01) + argmax over classes + exact
forward-fill of stop tokens (==500) done on host.

Sharding: data-parallel over batch B=16 across 8 cores (2 batches/core),
weights replicated. Per core 4000 tokens padded to 4096 = 8 chunks x 512.

Device layout: features on partitions, tokens on free axis. x is
transposed on host so DMA loads are contiguous. Matmuls run in fp32r
(full-rate PE). Layer-3 flips orientation (stationary = H2T token tile,
moving = W3) so logits land [tokens, 501] in PSUM for vector argmax.
"""

import numpy as np

import concourse.bass as bass
import concourse.mybir as mybir
import concourse.tile as tile
from concourse import bacc
from concourse.bass_utils import run_bass_kernel_spmd

B, T, DIM, H1, H2, OUT = 16, 2000, 1024, 894, 763, 501
OUTP = 512  # class dim padded for fp32r ISA (even/aligned free dim)
VOCAB = 500
MAX_ITERS = 10000
NCORES = 8
TOK = 4096          # padded tokens per core (4000 real)
REAL_TOK = 4000
NCHUNK = 8
CH = 512            # tokens per chunk
NSUB = TOK // 128   # 32 code columns

F32 = mybir.dt.float32
F32R = mybir.dt.float32r
MM_DT = F32R        # fp32r: full-rate PE, ~1e-4 matmul precision

_CACHE = {}


def _ceil(a, b):
    return (a + b - 1) // b


def build_kernel():
    nc = bacc.Bacc(target_bir_lowering=False)

    xT = nc.dram_tensor("xT", [DIM, TOK], MM_DT, kind="ExternalInput")
    W1d = nc.dram_tensor("W1", [DIM, H1], MM_DT, kind="ExternalInput")
    W2d = nc.dram_tensor("W2", [H1, H2], MM_DT, kind="ExternalInput")
    W3d = nc.dram_tensor("W3", [H2, OUTP], MM_DT, kind="ExternalInput")
    b1d = nc.dram_tensor("b1", [128, 7], F32, kind="ExternalInput")
    b2d = nc.dram_tensor("b2", [128, 6], F32, kind="ExternalInput")
    b3d = nc.dram_tensor("b3", [1, OUTP], MM_DT, kind="ExternalInput")
    codes_d = nc.dram_tensor("codes", [128, NSUB], mybir.dt.int32,
                             kind="ExternalOutput")
    gaps_d = nc.dram_tensor("gaps", [128, NSUB], F32, kind="ExternalOutput")

    KC1 = _ceil(DIM, 128)   # 8 (exact)
    KC2 = _ceil(H1, 128)    # 7, last 126
    KC3 = _ceil(H2, 128)    # 6, last 123
    MT1 = _ceil(H1, 128)    # 7, last 126
    MT2 = _ceil(H2, 128)    # 6, last 123

    LR = mybir.ActivationFunctionType.Lrelu

    with tile.TileContext(nc) as tc:
        with (
            tc.tile_pool(name="wpool", bufs=1) as wp,
            tc.tile_pool(name="xpool", bufs=3) as xp,
            tc.tile_pool(name="hpool", bufs=2) as hp,
            tc.tile_pool(name="spool", bufs=3) as sp,
            tc.tile_pool(name="cpool", bufs=1) as cp,
            tc.tile_pool(name="ps12", bufs=4, space="PSUM") as ps12,
            tc.tile_pool(name="ps3", bufs=3, space="PSUM") as ps3,
        ):
            # ---- weights / biases (loaded once) ----
            w1 = wp.tile([128, KC1, H1], MM_DT)
            nc.sync.dma_start(
                out=w1, in_=W1d[:].rearrange("(kc p) m -> p kc m", p=128))
            w2 = wp.tile([128, KC2, H2], MM_DT)
            nc.sync.dma_start(
                out=w2[:, 0:6, :],
                in_=W2d[0:768, :].rearrange("(kc p) m -> p kc m", p=128))
            nc.sync.dma_start(
                out=w2[0:126, 6, :], in_=W2d[768:894, :])
            w3 = wp.tile([128, KC3, OUTP], MM_DT)
            nc.sync.dma_start(
                out=w3[:, 0:5, :],
                in_=W3d[0:640, :].rearrange("(kc p) m -> p kc m", p=128))
            nc.sync.dma_start(
                out=w3[0:123, 5, :], in_=W3d[640:763, :])
            b1 = wp.tile([128, 7], F32)
            nc.sync.dma_start(out=b1, in_=b1d[:])
            b2 = wp.tile([128, 6], F32)
            nc.sync.dma_start(out=b2, in_=b2d[:])
            b3 = wp.tile([1, OUTP], MM_DT)
            nc.sync.dma_start(out=b3, in_=b3d[:])
            ones_f = wp.tile([1, 128], F32)
            nc.vector.memset(ones_f, 1.0)
            ones = wp.tile([1, 128], MM_DT)
            nc.vector.tensor_copy(ones, ones_f)

            codes_sb = cp.tile([128, NSUB], mybir.dt.int32)
            gaps_sb = cp.tile([128, NSUB], F32)

            for c in range(NCHUNK):
                xs = xp.tile([128, KC1, CH], MM_DT, tag="xslab")
                nc.sync.dma_start(
                    out=xs,
                    in_=xT[:, c * CH:(c + 1) * CH].rearrange(
                        "(kc p) t -> p kc t", p=128))

                # ---- layer 1: h1T[m*128+p, t] ----
                h1t = hp.tile([128, KC2, CH], MM_DT, tag="h1t")
                for mt in range(MT1):
                    m0 = mt * 128
                    mw = min(128, H1 - m0)
                    pt = ps12.tile([128, CH], F32, tag="pmm")
                    for kc in range(KC1):
                        nc.tensor.matmul(
                            pt[:mw, :], w1[:, kc, m0:m0 + mw], xs[:, kc, :],
                            start=(kc == 0), stop=(kc == KC1 - 1))
                    nc.scalar.activation(
                        h1t[:mw, mt, :], pt[:mw, :], LR,
                        bias=b1[:mw, mt:mt + 1], scale=1.0, alpha=0.01)

                # ---- layer 2 ----
                h2t = hp.tile([128, KC3, CH], MM_DT, tag="h2t")
                for mt in range(MT2):
                    m0 = mt * 128
                    mw = min(128, H2 - m0)
                    pt = ps12.tile([128, CH], F32, tag="pmm")
                    for kc in range(KC2):
                        kw = min(128, H1 - kc * 128)
                        nc.tensor.matmul(
                            pt[:mw, :], w2[:kw, kc, m0:m0 + mw],
                            h1t[:kw, kc, :],
                            start=(kc == 0), stop=(kc == KC2 - 1))
                    nc.scalar.activation(
                        h2t[:mw, mt, :], pt[:mw, :], LR,
                        bias=b2[:mw, mt:mt + 1], scale=1.0, alpha=0.01)

                # ---- layer 3 + argmax: per 128-token subtile ----
                for s in range(4):
                    t0 = s * 128
                    pl = ps3.tile([128, OUTP], F32, tag="plog")
                    nc.tensor.matmul(pl, ones, b3, start=True, stop=False)
                    for kc in range(KC3):
                        kw = min(128, H2 - kc * 128)
                        nc.tensor.matmul(
                            pl, h2t[:kw, kc, t0:t0 + 128], w3[:kw, kc, :],
                            start=False, stop=(kc == KC3 - 1))
                    logit = sp.tile([128, OUTP], F32, tag="logit")
                    nc.scalar.copy(logit, pl)
                    mx8 = sp.tile([128, 8], F32, tag="mx8")
                    ix8 = sp.tile([128, 8], mybir.dt.uint32, tag="ix8")
                    nc.vector.max(mx8, logit)
                    nc.vector.max_index(ix8, mx8, logit)
                    col = c * 4 + s
                    nc.vector.tensor_copy(
                        codes_sb.bitcast(mybir.dt.uint32)[:, col:col + 1],
                        ix8[:, 0:1])
                    nc.vector.tensor_sub(
                        gaps_sb[:, col:col + 1], mx8[:, 0:1], mx8[:, 1:2])

            nc.sync.dma_start(out=codes_d[:], in_=codes_sb)
            nc.sync.dma_start(out=gaps_d[:], in_=gaps_sb)

    nc.finalize()
    return nc


def _forward_fill_exact(code_flat: np.ndarray) -> np.ndarray:
    """Exact equivalent of the reference jax while-loop fill."""
    n = code_flat.shape[0]
    mask = code_flat == VOCAB
    if not mask.any():
        return code_flat
    if mask.all():
        return code_flat
    idx = np.where(~mask, np.arange(n), -1)
    fill = np.maximum.accumulate(idx)
    # wrap-around: positions before first non-stop take the last non-stop
    last = np.max(idx)
    dist = np.arange(n) - fill
    wrapped = fill < 0
    fill = np.where(wrapped, last, fill)
    dist = np.where(wrapped, np.arange(n) + (n - last), dist)
    out = code_flat[fill]
    # faithful MAX_ITERS cap: stops further than MAX_ITERS remain
    out = np.where(mask & (dist > MAX_ITERS), VOCAB, out)
    out = np.where(mask, out, code_flat)
    return out.astype(np.int32)


def kernel(x, W1, b1, W2, b2, W3, b3):
    x = np.asarray(x, dtype=np.float32)
    W1 = np.ascontiguousarray(np.asarray(W1, dtype=np.float32))
    W2 = np.ascontiguousarray(np.asarray(W2, dtype=np.float32))
    W3 = np.ascontiguousarray(np.asarray(W3, dtype=np.float32))
    b1 = np.asarray(b1, dtype=np.float32)
    b2 = np.asarray(b2, dtype=np.float32)
    b3 = np.asarray(b3, dtype=np.float32)

    if "nc" not in _CACHE:
        _CACHE["nc"] = build_kernel()
    nc = _CACHE["nc"]

    b1p = np.zeros((7 * 128,), np.float32)
    b1p[:H1] = b1
    b1p = np.ascontiguousarray(b1p.reshape(7, 128).T)
    b2p = np.zeros((6 * 128,), np.float32)
    b2p[:H2] = b2
    b2p = np.ascontiguousarray(b2p.reshape(6, 128).T)
    b3p = np.full((1, OUTP), -1e30, np.float32)
    b3p[0, :OUT] = b3
    W3p = np.zeros((H2, OUTP), np.float32)
    W3p[:, :OUT] = W3

    # one vectorized pad+transpose pass for all shards
    xa = np.zeros((NCORES, TOK, DIM), np.float32)
    xa[:, :REAL_TOK] = x.reshape(NCORES, REAL_TOK, DIM)
    xTa = np.ascontiguousarray(xa.transpose(0, 2, 1))
    in_maps = []
    for i in range(NCORES):
        in_maps.append({
            "xT": xTa[i], "W1": W1, "W2": W2, "W3": W3p,
            "b1": b1p, "b2": b2p, "b3": b3p,
        })

    _CACHE["in_maps"] = in_maps
    res = None
    for attempt in range(3):
        try:
            res = run_bass_kernel_spmd(nc, in_maps, core_ids=list(range(NCORES)))
            break
        except Exception:
            # transient NRT exec-unit wedge: cool down, then retry
            if attempt == 2:
                raise
            import time as _time
            _time.sleep(10)

    parts, gparts = [], []
    for i in range(NCORES):
        codes = res.results[i]["codes"]          # [128, 32]
        parts.append(codes.T.reshape(-1)[:REAL_TOK])   # token t = s*128+p
        gparts.append(res.results[i]["gaps"].T.reshape(-1)[:REAL_TOK])
    code = np.concatenate(parts).astype(np.int32)   # [32000]
    gap = np.concatenate(gparts).astype(np.float32)

    # fp32r argmax can flip near-ties; recompute uncertain tokens exactly
    unc = np.flatnonzero(gap < 1e-2)
    if unc.size:
        xf = x.reshape(-1, DIM)[unc].astype(np.float32)
        h = xf @ W1 + b1
        h = np.where(h >= 0, h, np.float32(0.01) * h).astype(np.float32)
        h = h @ W2 + b2
        h = np.where(h >= 0, h, np.float32(0.01) * h).astype(np.float32)
        lg = h @ W3 + b3
        code[unc] = np.argmax(lg, axis=-1).astype(np.int32)

    code = _forward_fill_exact(code)
    return code.reshape(B, T)



# revision 4
# speedup vs baseline: 68.8731x; 68.8731x over previous
"""Trainium2 Bass kernel for nn_LinearQuantizerModel.

MLP 1024->894->763->501 (leaky_relu 0.01) + argmax over classes + exact
forward-fill of stop tokens (==500) done on host.

Sharding: data-parallel over batch B=16 across 8 cores (2 batches/core),
weights replicated. Per core 4000 tokens padded to 4096 = 8 chunks x 512.

Device layout: features on partitions, tokens on free axis. x is
transposed on host so DMA loads are contiguous. Matmuls run in fp32r
(full-rate PE). Layer-3 flips orientation (stationary = H2T token tile,
moving = W3) so logits land [tokens, 501] in PSUM for vector argmax.

Execution: inputs are staged on the 8 devices once (device_put with a
"core"-sharded mesh) and the bass module is AOT-compiled once into a
fast-dispatch jax Compiled. Steady-state execution is then a single
async dispatch + one batched device_get of the two small outputs —
one tunnel roundtrip instead of re-uploading ~200 MB per call the way
per-call run_bass_kernel_spmd does. run_bass_kernel_spmd remains as a
fallback path.
"""

import numpy as np

import concourse.bass as bass
import concourse.mybir as mybir
import concourse.tile as tile
from concourse import bacc
from concourse.bass_utils import run_bass_kernel_spmd

B, T, DIM, H1, H2, OUT = 16, 2000, 1024, 894, 763, 501
OUTP = 512  # class dim padded for fp32r ISA (even/aligned free dim)
VOCAB = 500
MAX_ITERS = 10000
NCORES = 8
TOK = 4096          # padded tokens per core (4000 real)
REAL_TOK = 4000
NCHUNK = 8
CH = 512            # tokens per chunk
NSUB = TOK // 128   # 32 code columns

F32 = mybir.dt.float32
F32R = mybir.dt.float32r
MM_DT = F32R        # fp32r: full-rate PE, ~1e-4 matmul precision

_CACHE = {}


def _ceil(a, b):
    return (a + b - 1) // b


def build_kernel():
    nc = bacc.Bacc(target_bir_lowering=False)

    xT = nc.dram_tensor("xT", [DIM, TOK], MM_DT, kind="ExternalInput")
    W1d = nc.dram_tensor("W1", [DIM, H1], MM_DT, kind="ExternalInput")
    W2d = nc.dram_tensor("W2", [H1, H2], MM_DT, kind="ExternalInput")
    W3d = nc.dram_tensor("W3", [H2, OUTP], MM_DT, kind="ExternalInput")
    b1d = nc.dram_tensor("b1", [128, 7], F32, kind="ExternalInput")
    b2d = nc.dram_tensor("b2", [128, 6], F32, kind="ExternalInput")
    b3d = nc.dram_tensor("b3", [1, OUTP], MM_DT, kind="ExternalInput")
    codes_d = nc.dram_tensor("codes", [128, NSUB], mybir.dt.int32,
                             kind="ExternalOutput")
    gaps_d = nc.dram_tensor("gaps", [128, NSUB], F32, kind="ExternalOutput")

    KC1 = _ceil(DIM, 128)   # 8 (exact)
    KC2 = _ceil(H1, 128)    # 7, last 126
    KC3 = _ceil(H2, 128)    # 6, last 123
    MT1 = _ceil(H1, 128)    # 7, last 126
    MT2 = _ceil(H2, 128)    # 6, last 123

    LR = mybir.ActivationFunctionType.Lrelu

    with tile.TileContext(nc) as tc:
        with (
            tc.tile_pool(name="wpool", bufs=1) as wp,
            tc.tile_pool(name="xpool", bufs=3) as xp,
            tc.tile_pool(name="hpool", bufs=2) as hp,
            tc.tile_pool(name="spool", bufs=3) as sp,
            tc.tile_pool(name="cpool", bufs=1) as cp,
            tc.tile_pool(name="ps12", bufs=4, space="PSUM") as ps12,
            tc.tile_pool(name="ps3", bufs=3, space="PSUM") as ps3,
        ):
            # ---- weights / biases (loaded once) ----
            w1 = wp.tile([128, KC1, H1], MM_DT)
            nc.sync.dma_start(
                out=w1, in_=W1d[:].rearrange("(kc p) m -> p kc m", p=128))
            w2 = wp.tile([128, KC2, H2], MM_DT)
            nc.sync.dma_start(
                out=w2[:, 0:6, :],
                in_=W2d[0:768, :].rearrange("(kc p) m -> p kc m", p=128))
            nc.sync.dma_start(
                out=w2[0:126, 6, :], in_=W2d[768:894, :])
            w3 = wp.tile([128, KC3, OUTP], MM_DT)
            nc.sync.dma_start(
                out=w3[:, 0:5, :],
                in_=W3d[0:640, :].rearrange("(kc p) m -> p kc m", p=128))
            nc.sync.dma_start(
                out=w3[0:123, 5, :], in_=W3d[640:763, :])
            b1 = wp.tile([128, 7], F32)
            nc.sync.dma_start(out=b1, in_=b1d[:])
            b2 = wp.tile([128, 6], F32)
            nc.sync.dma_start(out=b2, in_=b2d[:])
            b3 = wp.tile([1, OUTP], MM_DT)
            nc.sync.dma_start(out=b3, in_=b3d[:])
            ones_f = wp.tile([1, 128], F32)
            nc.vector.memset(ones_f, 1.0)
            ones = wp.tile([1, 128], MM_DT)
            nc.vector.tensor_copy(ones, ones_f)

            codes_sb = cp.tile([128, NSUB], mybir.dt.int32)
            gaps_sb = cp.tile([128, NSUB], F32)

            for c in range(NCHUNK):
                xs = xp.tile([128, KC1, CH], MM_DT, tag="xslab")
                nc.sync.dma_start(
                    out=xs,
                    in_=xT[:, c * CH:(c + 1) * CH].rearrange(
                        "(kc p) t -> p kc t", p=128))

                # ---- layer 1: h1T[m*128+p, t] ----
                h1t = hp.tile([128, KC2, CH], MM_DT, tag="h1t")
                for mt in range(MT1):
                    m0 = mt * 128
                    mw = min(128, H1 - m0)
                    pt = ps12.tile([128, CH], F32, tag="pmm")
                    for kc in range(KC1):
                        nc.tensor.matmul(
                            pt[:mw, :], w1[:, kc, m0:m0 + mw], xs[:, kc, :],
                            start=(kc == 0), stop=(kc == KC1 - 1))
                    nc.scalar.activation(
                        h1t[:mw, mt, :], pt[:mw, :], LR,
                        bias=b1[:mw, mt:mt + 1], scale=1.0, alpha=0.01)

                # ---- layer 2 ----
                h2t = hp.tile([128, KC3, CH], MM_DT, tag="h2t")
                for mt in range(MT2):
                    m0 = mt * 128
                    mw = min(128, H2 - m0)
                    pt = ps12.tile([128, CH], F32, tag="pmm")
                    for kc in range(KC2):
                        kw = min(128, H1 - kc * 128)
                        nc.tensor.matmul(
                            pt[:mw, :], w2[:kw, kc, m0:m0 + mw],
                            h1t[:kw, kc, :],
                            start=(kc == 0), stop=(kc == KC2 - 1))
                    nc.scalar.activation(
                        h2t[:mw, mt, :], pt[:mw, :], LR,
                        bias=b2[:mw, mt:mt + 1], scale=1.0, alpha=0.01)

                # ---- layer 3 + argmax: per 128-token subtile ----
                for s in range(4):
                    t0 = s * 128
                    pl = ps3.tile([128, OUTP], F32, tag="plog")
                    nc.tensor.matmul(pl, ones, b3, start=True, stop=False)
                    for kc in range(KC3):
                        kw = min(128, H2 - kc * 128)
                        nc.tensor.matmul(
                            pl, h2t[:kw, kc, t0:t0 + 128], w3[:kw, kc, :],
                            start=False, stop=(kc == KC3 - 1))
                    logit = sp.tile([128, OUTP], F32, tag="logit")
                    nc.scalar.copy(logit, pl)
                    mx8 = sp.tile([128, 8], F32, tag="mx8")
                    ix8 = sp.tile([128, 8], mybir.dt.uint32, tag="ix8")
                    nc.vector.max(mx8, logit)
                    nc.vector.max_index(ix8, mx8, logit)
                    col = c * 4 + s
                    nc.vector.tensor_copy(
                        codes_sb.bitcast(mybir.dt.uint32)[:, col:col + 1],
                        ix8[:, 0:1])
                    nc.vector.tensor_sub(
                        gaps_sb[:, col:col + 1], mx8[:, 0:1], mx8[:, 1:2])

            nc.sync.dma_start(out=codes_d[:], in_=codes_sb)
            nc.sync.dma_start(out=gaps_d[:], in_=gaps_sb)

    nc.finalize()
    return nc


# ---------------------------------------------------------------------------
# Staged executor: device-resident inputs + AOT-compiled fast-dispatch call.
# ---------------------------------------------------------------------------

def _build_staged(nc, in_maps):
    """Stage per-core inputs on the 8 devices and AOT-compile the bass
    module into a reusable jax Compiled. Returns an execute() closure that
    runs one full 8-core kernel execution and returns {name: np.ndarray}
    per core, at a cost of one async dispatch + one batched device fetch."""
    import jax
    from jax.sharding import Mesh, PartitionSpec, NamedSharding
    from jax.experimental.shard_map import shard_map
    from concourse.bass2jax import (_bass_exec_p, install_neuronx_cc_hook,
                                    fast_dispatch_compile,
                                    partition_id_tensor)

    install_neuronx_cc_hook()

    partition_name = (nc.partition_id_tensor.name
                      if nc.partition_id_tensor else None)
    in_names, out_names, out_avals, zero_outs = [], [], [], []
    for alloc in nc.m.functions[0].allocations:
        if not isinstance(alloc, mybir.MemoryLocationSet):
            continue
        name = alloc.memorylocations[0].name
        if alloc.kind == "ExternalInput":
            if name != partition_name:
                in_names.append(name)
        elif alloc.kind == "ExternalOutput":
            out_names.append(name)
            shape = tuple(alloc.tensor_shape)
            dtype = mybir.dt.np(alloc.dtype)
            out_avals.append(jax.core.ShapedArray(shape, dtype))
            zero_outs.append(np.zeros(shape, dtype))
    n_params = len(in_names)
    all_names = list(in_names) + list(out_names)
    if partition_name is not None:
        all_names.append(partition_name)

    def _body(*args):
        operands = list(args)
        if partition_name is not None:
            operands.append(partition_id_tensor())
        outs = _bass_exec_p.bind(
            *operands,
            out_avals=tuple(out_avals),
            in_names=tuple(all_names),
            out_names=tuple(out_names),
            lowering_input_output_aliases=(),
            sim_require_finite=True,
            sim_require_nnan=True,
            nc=nc,
        )
        return tuple(outs)

    devices = jax.devices()[:NCORES]
    mesh = Mesh(np.asarray(devices), ("core",))
    sharding = NamedSharding(mesh, PartitionSpec("core"))
    in_specs = (PartitionSpec("core"),) * (n_params + len(out_names))
    out_specs = (PartitionSpec("core"),) * len(out_names)

    concat_in = [np.concatenate([np.asarray(m[n]) for m in in_maps], axis=0)
                 for n in in_names]
    concat_zero = [np.zeros((NCORES * z.shape[0], *z.shape[1:]), z.dtype)
                   for z in zero_outs]
    dev_in = [jax.device_put(a, sharding) for a in concat_in]
    dev_zero = [jax.device_put(a, sharding) for a in concat_zero]
    for a in dev_in + dev_zero:
        a.block_until_ready()

    fn = shard_map(_body, mesh=mesh, in_specs=in_specs, out_specs=out_specs,
                   check_rep=False)

    def _compile_plain():
        return jax.jit(fn, keep_unused=True).lower(*dev_in, *dev_zero).compile()

    try:
        compiled = fast_dispatch_compile(_compile_plain)
    except Exception:
        compiled = _compile_plain()

    def execute():
        outs = compiled(*dev_in, *dev_zero)
        host = jax.device_get(tuple(outs))  # one batched roundtrip
        return [
            {name: np.asarray(host[i]).reshape(NCORES, *out_avals[i].shape)[c]
             for i, name in enumerate(out_names)}
            for c in range(NCORES)
        ]

    return execute


def _forward_fill_exact(code_flat: np.ndarray) -> np.ndarray:
    """Exact equivalent of the reference jax while-loop fill."""
    n = code_flat.shape[0]
    mask = code_flat == VOCAB
    if not mask.any():
        return code_flat
    if mask.all():
        return code_flat
    idx = np.where(~mask, np.arange(n), -1)
    fill = np.maximum.accumulate(idx)
    # wrap-around: positions before first non-stop take the last non-stop
    last = np.max(idx)
    dist = np.arange(n) - fill
    wrapped = fill < 0
    fill = np.where(wrapped, last, fill)
    dist = np.where(wrapped, np.arange(n) + (n - last), dist)
    out = code_flat[fill]
    # faithful MAX_ITERS cap: stops further than MAX_ITERS remain
    out = np.where(mask & (dist > MAX_ITERS), VOCAB, out)
    out = np.where(mask, out, code_flat)
    return out.astype(np.int32)


def kernel(x, W1, b1, W2, b2, W3, b3):
    x = np.asarray(x, dtype=np.float32)
    W1 = np.ascontiguousarray(np.asarray(W1, dtype=np.float32))
    W2 = np.ascontiguousarray(np.asarray(W2, dtype=np.float32))
    W3 = np.ascontiguousarray(np.asarray(W3, dtype=np.float32))
    b1 = np.asarray(b1, dtype=np.float32)
    b2 = np.asarray(b2, dtype=np.float32)
    b3 = np.asarray(b3, dtype=np.float32)

    if "nc" not in _CACHE:
        _CACHE["nc"] = build_kernel()
    nc = _CACHE["nc"]

    b1p = np.zeros((7 * 128,), np.float32)
    b1p[:H1] = b1
    b1p = np.ascontiguousarray(b1p.reshape(7, 128).T)
    b2p = np.zeros((6 * 128,), np.float32)
    b2p[:H2] = b2
    b2p = np.ascontiguousarray(b2p.reshape(6, 128).T)
    b3p = np.full((1, OUTP), -1e30, np.float32)
    b3p[0, :OUT] = b3
    W3p = np.zeros((H2, OUTP), np.float32)
    W3p[:, :OUT] = W3

    # one vectorized pad+transpose pass for all shards
    xa = np.zeros((NCORES, TOK, DIM), np.float32)
    xa[:, :REAL_TOK] = x.reshape(NCORES, REAL_TOK, DIM)
    xTa = np.ascontiguousarray(xa.transpose(0, 2, 1))
    in_maps = []
    for i in range(NCORES):
        in_maps.append({
            "xT": xTa[i], "W1": W1, "W2": W2, "W3": W3p,
            "b1": b1p, "b2": b2p, "b3": b3p,
        })
    _CACHE["in_maps"] = in_maps

    results = None
    try:
        # restage every call: the staged device arrays must reflect THESE
        # inputs (a cached executor would replay the previous call's data)
        execute = _build_staged(nc, in_maps)
        _CACHE["execute"] = execute
        for attempt in range(3):
            try:
                results = execute()
                break
            except Exception:
                if attempt == 2:
                    raise
                import time as _time
                _time.sleep(5)
    except Exception:
        _CACHE.pop("execute", None)
        for attempt in range(3):
            try:
                res = run_bass_kernel_spmd(nc, in_maps,
                                           core_ids=list(range(NCORES)))
                results = res.results
                break
            except Exception:
                # transient NRT exec-unit wedge: cool down, then retry
                if attempt == 2:
                    raise
                import time as _time
                _time.sleep(10)

    parts, gparts = [], []
    for i in range(NCORES):
        codes = results[i]["codes"]          # [128, 32]
        parts.append(codes.T.reshape(-1)[:REAL_TOK])   # token t = s*128+p
        gparts.append(results[i]["gaps"].T.reshape(-1)[:REAL_TOK])
    code = np.concatenate(parts).astype(np.int32)   # [32000]
    gap = np.concatenate(gparts).astype(np.float32)

    # fp32r argmax can flip near-ties; recompute uncertain tokens exactly
    unc = np.flatnonzero(gap < 1e-2)
    if unc.size:
        xf = x.reshape(-1, DIM)[unc].astype(np.float32)
        h = xf @ W1 + b1
        h = np.where(h >= 0, h, np.float32(0.01) * h).astype(np.float32)
        h = h @ W2 + b2
        h = np.where(h >= 0, h, np.float32(0.01) * h).astype(np.float32)
        lg = h @ W3 + b3
        code[unc] = np.argmax(lg, axis=-1).astype(np.int32)

    code = _forward_fill_exact(code)
    return code.reshape(B, T)


# revision 5
# speedup vs baseline: 6078.8602x; 88.2618x over previous
"""Trainium2 Bass kernel for nn_LinearQuantizerModel.

MLP 1024->894->763->501 (leaky_relu 0.01) + argmax over classes + exact
forward-fill of stop tokens (==500) done on host.

Sharding: data-parallel over batch B=16 across 8 cores (2 batches/core),
weights replicated. Per core 4000 tokens padded to 4096 = 8 chunks x 512.

Device layout: features on partitions, tokens on free axis. x is
transposed on host so DMA loads are contiguous. Matmuls run in fp32r
(full-rate PE). Layer-3 flips orientation (stationary = H2T token tile,
moving = W3) so logits land [tokens, 501] in PSUM for vector argmax.

Execution: inputs are staged on the 8 devices once (device_put with a
"core"-sharded mesh) and the bass module is AOT-compiled once into a
fast-dispatch jax Compiled. Steady-state execution is then a single
async dispatch + one batched device_get of the two small outputs —
one tunnel roundtrip instead of re-uploading ~200 MB per call the way
per-call run_bass_kernel_spmd does. run_bass_kernel_spmd remains as a
fallback path.
"""

import numpy as np

import concourse.bass as bass
import concourse.mybir as mybir
import concourse.tile as tile
from concourse import bacc
from concourse.bass_utils import run_bass_kernel_spmd

B, T, DIM, H1, H2, OUT = 16, 2000, 1024, 894, 763, 501
OUTP = 512  # class dim padded for fp32r ISA (even/aligned free dim)
VOCAB = 500
MAX_ITERS = 10000
NCORES = 8
TOK = 4096          # padded tokens per core (4000 real)
REAL_TOK = 4000
NCHUNK = 8
CH = 512            # tokens per chunk
NSUB = TOK // 128   # 32 code columns

F32 = mybir.dt.float32
F32R = mybir.dt.float32r
MM_DT = F32R        # fp32r: full-rate PE, ~1e-4 matmul precision

_CACHE = {}


def _ceil(a, b):
    return (a + b - 1) // b


def build_kernel():
    nc = bacc.Bacc(target_bir_lowering=False)

    xT = nc.dram_tensor("xT", [DIM, TOK], MM_DT, kind="ExternalInput")
    W1d = nc.dram_tensor("W1", [DIM, H1], MM_DT, kind="ExternalInput")
    W2d = nc.dram_tensor("W2", [H1, H2], MM_DT, kind="ExternalInput")
    W3d = nc.dram_tensor("W3", [H2, OUTP], MM_DT, kind="ExternalInput")
    b1d = nc.dram_tensor("b1", [128, 7], F32, kind="ExternalInput")
    b2d = nc.dram_tensor("b2", [128, 6], F32, kind="ExternalInput")
    b3d = nc.dram_tensor("b3", [1, OUTP], MM_DT, kind="ExternalInput")
    codes_d = nc.dram_tensor("codes", [128, NSUB], mybir.dt.int32,
                             kind="ExternalOutput")
    gaps_d = nc.dram_tensor("gaps", [128, NSUB], F32, kind="ExternalOutput")

    KC1 = _ceil(DIM, 128)   # 8 (exact)
    KC2 = _ceil(H1, 128)    # 7, last 126
    KC3 = _ceil(H2, 128)    # 6, last 123
    MT1 = _ceil(H1, 128)    # 7, last 126
    MT2 = _ceil(H2, 128)    # 6, last 123

    LR = mybir.ActivationFunctionType.Lrelu

    with tile.TileContext(nc) as tc:
        with (
            tc.tile_pool(name="wpool", bufs=1) as wp,
            tc.tile_pool(name="xpool", bufs=3) as xp,
            tc.tile_pool(name="hpool", bufs=2) as hp,
            tc.tile_pool(name="spool", bufs=3) as sp,
            tc.tile_pool(name="cpool", bufs=1) as cp,
            tc.tile_pool(name="ps12", bufs=4, space="PSUM") as ps12,
            tc.tile_pool(name="ps3", bufs=3, space="PSUM") as ps3,
        ):
            # ---- weights / biases (loaded once) ----
            w1 = wp.tile([128, KC1, H1], MM_DT)
            nc.sync.dma_start(
                out=w1, in_=W1d[:].rearrange("(kc p) m -> p kc m", p=128))
            w2 = wp.tile([128, KC2, H2], MM_DT)
            nc.sync.dma_start(
                out=w2[:, 0:6, :],
                in_=W2d[0:768, :].rearrange("(kc p) m -> p kc m", p=128))
            nc.sync.dma_start(
                out=w2[0:126, 6, :], in_=W2d[768:894, :])
            w3 = wp.tile([128, KC3, OUTP], MM_DT)
            nc.sync.dma_start(
                out=w3[:, 0:5, :],
                in_=W3d[0:640, :].rearrange("(kc p) m -> p kc m", p=128))
            nc.sync.dma_start(
                out=w3[0:123, 5, :], in_=W3d[640:763, :])
            b1 = wp.tile([128, 7], F32)
            nc.sync.dma_start(out=b1, in_=b1d[:])
            b2 = wp.tile([128, 6], F32)
            nc.sync.dma_start(out=b2, in_=b2d[:])
            b3 = wp.tile([1, OUTP], MM_DT)
            nc.sync.dma_start(out=b3, in_=b3d[:])
            ones_f = wp.tile([1, 128], F32)
            nc.vector.memset(ones_f, 1.0)
            ones = wp.tile([1, 128], MM_DT)
            nc.vector.tensor_copy(ones, ones_f)

            codes_sb = cp.tile([128, NSUB], mybir.dt.int32)
            gaps_sb = cp.tile([128, NSUB], F32)

            for c in range(NCHUNK):
                xs = xp.tile([128, KC1, CH], MM_DT, tag="xslab")
                nc.sync.dma_start(
                    out=xs,
                    in_=xT[:, c * CH:(c + 1) * CH].rearrange(
                        "(kc p) t -> p kc t", p=128))

                # ---- layer 1: h1T[m*128+p, t] ----
                h1t = hp.tile([128, KC2, CH], MM_DT, tag="h1t")
                for mt in range(MT1):
                    m0 = mt * 128
                    mw = min(128, H1 - m0)
                    pt = ps12.tile([128, CH], F32, tag="pmm")
                    for kc in range(KC1):
                        nc.tensor.matmul(
                            pt[:mw, :], w1[:, kc, m0:m0 + mw], xs[:, kc, :],
                            start=(kc == 0), stop=(kc == KC1 - 1))
                    nc.scalar.activation(
                        h1t[:mw, mt, :], pt[:mw, :], LR,
                        bias=b1[:mw, mt:mt + 1], scale=1.0, alpha=0.01)

                # ---- layer 2 ----
                h2t = hp.tile([128, KC3, CH], MM_DT, tag="h2t")
                for mt in range(MT2):
                    m0 = mt * 128
                    mw = min(128, H2 - m0)
                    pt = ps12.tile([128, CH], F32, tag="pmm")
                    for kc in range(KC2):
                        kw = min(128, H1 - kc * 128)
                        nc.tensor.matmul(
                            pt[:mw, :], w2[:kw, kc, m0:m0 + mw],
                            h1t[:kw, kc, :],
                            start=(kc == 0), stop=(kc == KC2 - 1))
                    nc.scalar.activation(
                        h2t[:mw, mt, :], pt[:mw, :], LR,
                        bias=b2[:mw, mt:mt + 1], scale=1.0, alpha=0.01)

                # ---- layer 3 + argmax: per 128-token subtile ----
                for s in range(4):
                    t0 = s * 128
                    pl = ps3.tile([128, OUTP], F32, tag="plog")
                    nc.tensor.matmul(pl, ones, b3, start=True, stop=False)
                    for kc in range(KC3):
                        kw = min(128, H2 - kc * 128)
                        nc.tensor.matmul(
                            pl, h2t[:kw, kc, t0:t0 + 128], w3[:kw, kc, :],
                            start=False, stop=(kc == KC3 - 1))
                    logit = sp.tile([128, OUTP], F32, tag="logit")
                    nc.scalar.copy(logit, pl)
                    mx8 = sp.tile([128, 8], F32, tag="mx8")
                    ix8 = sp.tile([128, 8], mybir.dt.uint32, tag="ix8")
                    nc.vector.max(mx8, logit)
                    nc.vector.max_index(ix8, mx8, logit)
                    col = c * 4 + s
                    nc.vector.tensor_copy(
                        codes_sb.bitcast(mybir.dt.uint32)[:, col:col + 1],
                        ix8[:, 0:1])
                    nc.vector.tensor_sub(
                        gaps_sb[:, col:col + 1], mx8[:, 0:1], mx8[:, 1:2])

            nc.sync.dma_start(out=codes_d[:], in_=codes_sb)
            nc.sync.dma_start(out=gaps_d[:], in_=gaps_sb)

    nc.finalize()
    return nc


# ---------------------------------------------------------------------------
# Staged executor: device-resident inputs + AOT-compiled fast-dispatch call.
# ---------------------------------------------------------------------------

def _build_staged(nc, in_maps):
    """Stage per-core inputs on the 8 devices and AOT-compile the bass
    module into a reusable jax Compiled. Returns an execute() closure that
    runs one full 8-core kernel execution and returns {name: np.ndarray}
    per core, at a cost of one async dispatch + one batched device fetch."""
    import jax
    from jax.sharding import Mesh, PartitionSpec, NamedSharding
    from jax.experimental.shard_map import shard_map
    from concourse.bass2jax import (_bass_exec_p, install_neuronx_cc_hook,
                                    fast_dispatch_compile,
                                    partition_id_tensor)

    install_neuronx_cc_hook()

    partition_name = (nc.partition_id_tensor.name
                      if nc.partition_id_tensor else None)
    in_names, out_names, out_avals, zero_outs = [], [], [], []
    for alloc in nc.m.functions[0].allocations:
        if not isinstance(alloc, mybir.MemoryLocationSet):
            continue
        name = alloc.memorylocations[0].name
        if alloc.kind == "ExternalInput":
            if name != partition_name:
                in_names.append(name)
        elif alloc.kind == "ExternalOutput":
            out_names.append(name)
            shape = tuple(alloc.tensor_shape)
            dtype = mybir.dt.np(alloc.dtype)
            out_avals.append(jax.core.ShapedArray(shape, dtype))
            zero_outs.append(np.zeros(shape, dtype))
    n_params = len(in_names)
    all_names = list(in_names) + list(out_names)
    if partition_name is not None:
        all_names.append(partition_name)

    def _body(*args):
        operands = list(args)
        if partition_name is not None:
            operands.append(partition_id_tensor())
        outs = _bass_exec_p.bind(
            *operands,
            out_avals=tuple(out_avals),
            in_names=tuple(all_names),
            out_names=tuple(out_names),
            lowering_input_output_aliases=(),
            sim_require_finite=True,
            sim_require_nnan=True,
            nc=nc,
        )
        return tuple(outs)

    devices = jax.devices()[:NCORES]
    mesh = Mesh(np.asarray(devices), ("core",))
    sharding = NamedSharding(mesh, PartitionSpec("core"))
    in_specs = (PartitionSpec("core"),) * (n_params + len(out_names))
    out_specs = (PartitionSpec("core"),) * len(out_names)

    concat_in = [np.concatenate([np.asarray(m[n]) for m in in_maps], axis=0)
                 for n in in_names]
    concat_zero = [np.zeros((NCORES * z.shape[0], *z.shape[1:]), z.dtype)
                   for z in zero_outs]
    dev_in = [jax.device_put(a, sharding) for a in concat_in]
    dev_zero = [jax.device_put(a, sharding) for a in concat_zero]
    for a in dev_in + dev_zero:
        a.block_until_ready()

    fn = shard_map(_body, mesh=mesh, in_specs=in_specs, out_specs=out_specs,
                   check_rep=False)

    def _compile_plain():
        return jax.jit(fn, keep_unused=True).lower(*dev_in, *dev_zero).compile()

    try:
        compiled = fast_dispatch_compile(_compile_plain)
    except Exception:
        compiled = _compile_plain()

    _CACHE["compiled"] = compiled
    _CACHE["dev_in"] = dev_in
    _CACHE["dev_zero"] = dev_zero

    def execute():
        outs = compiled(*dev_in, *dev_zero)
        host = jax.device_get(tuple(outs))  # one batched roundtrip
        return [
            {name: np.asarray(host[i]).reshape(NCORES, *out_avals[i].shape)[c]
             for i, name in enumerate(out_names)}
            for c in range(NCORES)
        ]

    return execute


def _forward_fill_exact(code_flat: np.ndarray) -> np.ndarray:
    """Exact equivalent of the reference jax while-loop fill."""
    n = code_flat.shape[0]
    mask = code_flat == VOCAB
    if not mask.any():
        return code_flat
    if mask.all():
        return code_flat
    idx = np.where(~mask, np.arange(n), -1)
    fill = np.maximum.accumulate(idx)
    # wrap-around: positions before first non-stop take the last non-stop
    last = np.max(idx)
    dist = np.arange(n) - fill
    wrapped = fill < 0
    fill = np.where(wrapped, last, fill)
    dist = np.where(wrapped, np.arange(n) + (n - last), dist)
    out = code_flat[fill]
    # faithful MAX_ITERS cap: stops further than MAX_ITERS remain
    out = np.where(mask & (dist > MAX_ITERS), VOCAB, out)
    out = np.where(mask, out, code_flat)
    return out.astype(np.int32)


def kernel(x, W1, b1, W2, b2, W3, b3):
    x = np.asarray(x, dtype=np.float32)
    W1 = np.ascontiguousarray(np.asarray(W1, dtype=np.float32))
    W2 = np.ascontiguousarray(np.asarray(W2, dtype=np.float32))
    W3 = np.ascontiguousarray(np.asarray(W3, dtype=np.float32))
    b1 = np.asarray(b1, dtype=np.float32)
    b2 = np.asarray(b2, dtype=np.float32)
    b3 = np.asarray(b3, dtype=np.float32)

    if "nc" not in _CACHE:
        _CACHE["nc"] = build_kernel()
    nc = _CACHE["nc"]

    b1p = np.zeros((7 * 128,), np.float32)
    b1p[:H1] = b1
    b1p = np.ascontiguousarray(b1p.reshape(7, 128).T)
    b2p = np.zeros((6 * 128,), np.float32)
    b2p[:H2] = b2
    b2p = np.ascontiguousarray(b2p.reshape(6, 128).T)
    b3p = np.full((1, OUTP), -1e30, np.float32)
    b3p[0, :OUT] = b3
    W3p = np.zeros((H2, OUTP), np.float32)
    W3p[:, :OUT] = W3

    # one vectorized pad+transpose pass for all shards
    xa = np.zeros((NCORES, TOK, DIM), np.float32)
    xa[:, :REAL_TOK] = x.reshape(NCORES, REAL_TOK, DIM)
    xTa = np.ascontiguousarray(xa.transpose(0, 2, 1))
    in_maps = []
    for i in range(NCORES):
        in_maps.append({
            "xT": xTa[i], "W1": W1, "W2": W2, "W3": W3p,
            "b1": b1p, "b2": b2p, "b3": b3p,
        })
    _CACHE["in_maps"] = in_maps

    results = None
    try:
        # restage every call: the staged device arrays must reflect THESE
        # inputs (a cached executor would replay the previous call's data)
        execute = _build_staged(nc, in_maps)
        _CACHE["execute"] = execute
        for attempt in range(3):
            try:
                results = execute()
                break
            except Exception:
                if attempt == 2:
                    raise
                import time as _time
                _time.sleep(5)
    except Exception:
        _CACHE.pop("execute", None)
        for attempt in range(3):
            try:
                res = run_bass_kernel_spmd(nc, in_maps,
                                           core_ids=list(range(NCORES)))
                results = res.results
                break
            except Exception:
                # transient NRT exec-unit wedge: cool down, then retry
                if attempt == 2:
                    raise
                import time as _time
                _time.sleep(10)

    parts, gparts = [], []
    for i in range(NCORES):
        codes = results[i]["codes"]          # [128, 32]
        parts.append(codes.T.reshape(-1)[:REAL_TOK])   # token t = s*128+p
        gparts.append(results[i]["gaps"].T.reshape(-1)[:REAL_TOK])
    code = np.concatenate(parts).astype(np.int32)   # [32000]
    gap = np.concatenate(gparts).astype(np.float32)

    # fp32r argmax can flip near-ties; recompute uncertain tokens exactly
    unc = np.flatnonzero(gap < 1e-2)
    if unc.size:
        xf = x.reshape(-1, DIM)[unc].astype(np.float32)
        h = xf @ W1 + b1
        h = np.where(h >= 0, h, np.float32(0.01) * h).astype(np.float32)
        h = h @ W2 + b2
        h = np.where(h >= 0, h, np.float32(0.01) * h).astype(np.float32)
        lg = h @ W3 + b3
        code[unc] = np.argmax(lg, axis=-1).astype(np.int32)

    code = _forward_fill_exact(code)
    return code.reshape(B, T)


# revision 6
# speedup vs baseline: 14276.7049x; 2.3486x over previous
"""Trainium2 Bass kernel for nn_LinearQuantizerModel.

MLP 1024->894->763->501 (leaky_relu 0.01) + argmax over classes + exact
forward-fill of stop tokens (==500) done on host.

Sharding: data-parallel over batch B=16 across 8 cores (2 batches/core),
weights replicated. Per core 4000 tokens padded to 4096 = 8 chunks x 512.

Device layout: features on partitions, tokens on free axis. x is
transposed on host so DMA loads are contiguous. Matmuls run in fp32r
(full-rate PE). Layer-3 flips orientation (stationary = H2T token tile,
moving = W3) so logits land [tokens, 501] in PSUM for vector argmax.

Execution: inputs are staged on the 8 devices once (device_put with a
"core"-sharded mesh) and the bass module is AOT-compiled once into a
fast-dispatch jax Compiled. Steady-state execution is then a single
async dispatch + one batched device_get of the two small outputs —
one tunnel roundtrip instead of re-uploading ~200 MB per call the way
per-call run_bass_kernel_spmd does. run_bass_kernel_spmd remains as a
fallback path.

build_kernel(nruns=N) additionally emits a benchmark variant whose whole
body (weight loads included) re-executes N times inside a tc.For_i loop;
one NEFF execution then runs the kernel N times back-to-back, letting
wall/N measure per-iteration device time with launch/tunnel overhead
amortized away (NTFF tracing is unavailable under axon).
"""

import contextlib

import numpy as np

import concourse.bass as bass
import concourse.mybir as mybir
import concourse.tile as tile
from concourse import bacc
from concourse.bass_utils import run_bass_kernel_spmd

B, T, DIM, H1, H2, OUT = 16, 2000, 1024, 894, 763, 501
OUTP = 512  # class dim padded for fp32r ISA (even/aligned free dim)
VOCAB = 500
MAX_ITERS = 10000
NCORES = 8
TOK = 4096          # padded tokens per core (4000 real)
REAL_TOK = 4000
NCHUNK = 8
CH = 512            # tokens per chunk
NSUB = TOK // 128   # 32 code columns

F32 = mybir.dt.float32
F32R = mybir.dt.float32r
MM_DT = F32R        # fp32r: full-rate PE, ~1e-4 matmul precision

_CACHE = {}


def _ceil(a, b):
    return (a + b - 1) // b


def build_kernel(nruns=1):
    nc = bacc.Bacc(target_bir_lowering=False)

    xT = nc.dram_tensor("xT", [DIM, TOK], MM_DT, kind="ExternalInput")
    W1d = nc.dram_tensor("W1", [DIM, H1], MM_DT, kind="ExternalInput")
    W2d = nc.dram_tensor("W2", [H1, H2], MM_DT, kind="ExternalInput")
    W3d = nc.dram_tensor("W3", [H2, OUTP], MM_DT, kind="ExternalInput")
    b1d = nc.dram_tensor("b1", [128, 7], F32, kind="ExternalInput")
    b2d = nc.dram_tensor("b2", [128, 6], F32, kind="ExternalInput")
    b3d = nc.dram_tensor("b3", [1, OUTP], MM_DT, kind="ExternalInput")
    codes_d = nc.dram_tensor("codes", [128, NSUB], mybir.dt.int32,
                             kind="ExternalOutput")
    gaps_d = nc.dram_tensor("gaps", [128, NSUB], F32, kind="ExternalOutput")

    KC1 = _ceil(DIM, 128)   # 8 (exact)
    KC2 = _ceil(H1, 128)    # 7, last 126
    KC3 = _ceil(H2, 128)    # 6, last 123
    MT1 = _ceil(H1, 128)    # 7, last 126
    MT2 = _ceil(H2, 128)    # 6, last 123

    LR = mybir.ActivationFunctionType.Lrelu

    with tile.TileContext(nc) as tc:
        with (
            tc.tile_pool(name="wpool", bufs=1) as wp,
            tc.tile_pool(name="xpool", bufs=3) as xp,
            tc.tile_pool(name="hpool", bufs=2) as hp,
            tc.tile_pool(name="spool", bufs=3) as sp,
            tc.tile_pool(name="cpool", bufs=1) as cp,
            tc.tile_pool(name="ps12", bufs=4, space="PSUM") as ps12,
            tc.tile_pool(name="ps3", bufs=3, space="PSUM") as ps3,
        ):
            loop = tc.For_i(0, nruns, 1) if nruns > 1 else (
                contextlib.nullcontext())
            with loop:
                # ---- weights / biases (loaded once per run) ----
                w1 = wp.tile([128, KC1, H1], MM_DT, tag="w1")
                nc.sync.dma_start(
                    out=w1, in_=W1d[:].rearrange("(kc p) m -> p kc m", p=128))
                w2 = wp.tile([128, KC2, H2], MM_DT, tag="w2")
                nc.sync.dma_start(
                    out=w2[:, 0:6, :],
                    in_=W2d[0:768, :].rearrange("(kc p) m -> p kc m", p=128))
                nc.sync.dma_start(
                    out=w2[0:126, 6, :], in_=W2d[768:894, :])
                w3 = wp.tile([128, KC3, OUTP], MM_DT, tag="w3")
                nc.sync.dma_start(
                    out=w3[:, 0:5, :],
                    in_=W3d[0:640, :].rearrange("(kc p) m -> p kc m", p=128))
                nc.sync.dma_start(
                    out=w3[0:123, 5, :], in_=W3d[640:763, :])
                b1 = wp.tile([128, 7], F32, tag="b1")
                nc.sync.dma_start(out=b1, in_=b1d[:])
                b2 = wp.tile([128, 6], F32, tag="b2")
                nc.sync.dma_start(out=b2, in_=b2d[:])
                b3 = wp.tile([1, OUTP], MM_DT, tag="b3")
                nc.sync.dma_start(out=b3, in_=b3d[:])
                ones_f = wp.tile([1, 128], F32, tag="ones_f")
                nc.vector.memset(ones_f, 1.0)
                ones = wp.tile([1, 128], MM_DT, tag="ones")
                nc.vector.tensor_copy(ones, ones_f)

                codes_sb = cp.tile([128, NSUB], mybir.dt.int32, tag="codes")
                gaps_sb = cp.tile([128, NSUB], F32, tag="gaps")

                for c in range(NCHUNK):
                    xs = xp.tile([128, KC1, CH], MM_DT, tag="xslab")
                    nc.sync.dma_start(
                        out=xs,
                        in_=xT[:, c * CH:(c + 1) * CH].rearrange(
                            "(kc p) t -> p kc t", p=128))

                    # ---- layer 1: h1T[m*128+p, t] ----
                    h1t = hp.tile([128, KC2, CH], MM_DT, tag="h1t")
                    for mt in range(MT1):
                        m0 = mt * 128
                        mw = min(128, H1 - m0)
                        pt = ps12.tile([128, CH], F32, tag="pmm")
                        for kc in range(KC1):
                            nc.tensor.matmul(
                                pt[:mw, :], w1[:, kc, m0:m0 + mw],
                                xs[:, kc, :],
                                start=(kc == 0), stop=(kc == KC1 - 1))
                        nc.scalar.activation(
                            h1t[:mw, mt, :], pt[:mw, :], LR,
                            bias=b1[:mw, mt:mt + 1], scale=1.0, alpha=0.01)

                    # ---- layer 2 ----
                    h2t = hp.tile([128, KC3, CH], MM_DT, tag="h2t")
                    for mt in range(MT2):
                        m0 = mt * 128
                        mw = min(128, H2 - m0)
                        pt = ps12.tile([128, CH], F32, tag="pmm")
                        for kc in range(KC2):
                            kw = min(128, H1 - kc * 128)
                            nc.tensor.matmul(
                                pt[:mw, :], w2[:kw, kc, m0:m0 + mw],
                                h1t[:kw, kc, :],
                                start=(kc == 0), stop=(kc == KC2 - 1))
                        nc.scalar.activation(
                            h2t[:mw, mt, :], pt[:mw, :], LR,
                            bias=b2[:mw, mt:mt + 1], scale=1.0, alpha=0.01)

                    # ---- layer 3 + argmax: per 128-token subtile ----
                    for s in range(4):
                        t0 = s * 128
                        pl = ps3.tile([128, OUTP], F32, tag="plog")
                        nc.tensor.matmul(pl, ones, b3, start=True, stop=False)
                        for kc in range(KC3):
                            kw = min(128, H2 - kc * 128)
                            nc.tensor.matmul(
                                pl, h2t[:kw, kc, t0:t0 + 128], w3[:kw, kc, :],
                                start=False, stop=(kc == KC3 - 1))
                        logit = sp.tile([128, OUTP], F32, tag="logit")
                        nc.scalar.copy(logit, pl)
                        mx8 = sp.tile([128, 8], F32, tag="mx8")
                        ix8 = sp.tile([128, 8], mybir.dt.uint32, tag="ix8")
                        nc.vector.max(mx8, logit)
                        nc.vector.max_index(ix8, mx8, logit)
                        col = c * 4 + s
                        nc.vector.tensor_copy(
                            codes_sb.bitcast(mybir.dt.uint32)[:, col:col + 1],
                            ix8[:, 0:1])
                        nc.vector.tensor_sub(
                            gaps_sb[:, col:col + 1], mx8[:, 0:1], mx8[:, 1:2])

                nc.sync.dma_start(out=codes_d[:], in_=codes_sb)
                nc.sync.dma_start(out=gaps_d[:], in_=gaps_sb)

    nc.finalize()
    return nc


# ---------------------------------------------------------------------------
# Staged executor: device-resident inputs + AOT-compiled fast-dispatch call.
# ---------------------------------------------------------------------------

def _introspect(nc):
    import jax
    partition_name = (nc.partition_id_tensor.name
                      if nc.partition_id_tensor else None)
    in_names, out_names, out_avals = [], [], []
    for alloc in nc.m.functions[0].allocations:
        if not isinstance(alloc, mybir.MemoryLocationSet):
            continue
        name = alloc.memorylocations[0].name
        if alloc.kind == "ExternalInput":
            if name != partition_name:
                in_names.append(name)
        elif alloc.kind == "ExternalOutput":
            out_names.append(name)
            out_avals.append(jax.core.ShapedArray(
                tuple(alloc.tensor_shape), mybir.dt.np(alloc.dtype)))
    return partition_name, in_names, out_names, out_avals


def _aot_compile(nc, dev_in, dev_zero):
    """AOT-compile nc into a reusable (fast-dispatch) jax Compiled taking
    (dev_in..., dev_zero...) sharded over the 8-core mesh."""
    import jax
    from jax.sharding import Mesh, PartitionSpec
    from jax.experimental.shard_map import shard_map
    from concourse.bass2jax import (_bass_exec_p, install_neuronx_cc_hook,
                                    fast_dispatch_compile,
                                    partition_id_tensor)

    install_neuronx_cc_hook()
    partition_name, in_names, out_names, out_avals = _introspect(nc)
    n_params = len(in_names)
    all_names = list(in_names) + list(out_names)
    if partition_name is not None:
        all_names.append(partition_name)

    def _body(*args):
        operands = list(args)
        if partition_name is not None:
            operands.append(partition_id_tensor())
        outs = _bass_exec_p.bind(
            *operands,
            out_avals=tuple(out_avals),
            in_names=tuple(all_names),
            out_names=tuple(out_names),
            lowering_input_output_aliases=(),
            sim_require_finite=True,
            sim_require_nnan=True,
            nc=nc,
        )
        return tuple(outs)

    devices = jax.devices()[:NCORES]
    mesh = Mesh(np.asarray(devices), ("core",))
    in_specs = (PartitionSpec("core"),) * (n_params + len(out_names))
    out_specs = (PartitionSpec("core"),) * len(out_names)
    fn = shard_map(_body, mesh=mesh, in_specs=in_specs, out_specs=out_specs,
                   check_rep=False)

    def _compile_plain():
        return jax.jit(fn, keep_unused=True).lower(
            *dev_in, *dev_zero).compile()

    try:
        return fast_dispatch_compile(_compile_plain)
    except Exception:
        return _compile_plain()


def _stage(nc, in_maps):
    """device_put the per-core inputs (concat over the core axis) + zeroed
    output buffers onto the 8-device mesh."""
    import jax
    from jax.sharding import Mesh, PartitionSpec, NamedSharding

    partition_name, in_names, out_names, out_avals = _introspect(nc)
    devices = jax.devices()[:NCORES]
    mesh = Mesh(np.asarray(devices), ("core",))
    sharding = NamedSharding(mesh, PartitionSpec("core"))

    concat_in = [np.concatenate([np.asarray(m[n]) for m in in_maps], axis=0)
                 for n in in_names]
    concat_zero = [np.zeros((NCORES * a.shape[0], *a.shape[1:]), a.dtype)
                   for a in out_avals]
    dev_in = [jax.device_put(a, sharding) for a in concat_in]
    dev_zero = [jax.device_put(a, sharding) for a in concat_zero]
    for a in dev_in + dev_zero:
        a.block_until_ready()
    return dev_in, dev_zero


def _build_staged(nc, in_maps):
    """Stage inputs + AOT-compile; returns execute() -> per-core result
    dicts at a cost of one async dispatch + one batched device fetch."""
    import jax

    _, _, out_names, out_avals = _introspect(nc)
    dev_in, dev_zero = _stage(nc, in_maps)
    compiled = _aot_compile(nc, dev_in, dev_zero)

    _CACHE["compiled"] = compiled
    _CACHE["dev_in"] = dev_in
    _CACHE["dev_zero"] = dev_zero

    def execute():
        outs = compiled(*dev_in, *dev_zero)
        host = jax.device_get(tuple(outs))  # one batched roundtrip
        return [
            {name: np.asarray(host[i]).reshape(NCORES, *out_avals[i].shape)[c]
             for i, name in enumerate(out_names)}
            for c in range(NCORES)
        ]

    return execute


def _forward_fill_exact(code_flat: np.ndarray) -> np.ndarray:
    """Exact equivalent of the reference jax while-loop fill."""
    n = code_flat.shape[0]
    mask = code_flat == VOCAB
    if not mask.any():
        return code_flat
    if mask.all():
        return code_flat
    idx = np.where(~mask, np.arange(n), -1)
    fill = np.maximum.accumulate(idx)
    # wrap-around: positions before first non-stop take the last non-stop
    last = np.max(idx)
    dist = np.arange(n) - fill
    wrapped = fill < 0
    fill = np.where(wrapped, last, fill)
    dist = np.where(wrapped, np.arange(n) + (n - last), dist)
    out = code_flat[fill]
    # faithful MAX_ITERS cap: stops further than MAX_ITERS remain
    out = np.where(mask & (dist > MAX_ITERS), VOCAB, out)
    out = np.where(mask, out, code_flat)
    return out.astype(np.int32)


def kernel(x, W1, b1, W2, b2, W3, b3):
    x = np.asarray(x, dtype=np.float32)
    W1 = np.ascontiguousarray(np.asarray(W1, dtype=np.float32))
    W2 = np.ascontiguousarray(np.asarray(W2, dtype=np.float32))
    W3 = np.ascontiguousarray(np.asarray(W3, dtype=np.float32))
    b1 = np.asarray(b1, dtype=np.float32)
    b2 = np.asarray(b2, dtype=np.float32)
    b3 = np.asarray(b3, dtype=np.float32)

    if "nc" not in _CACHE:
        _CACHE["nc"] = build_kernel()
    nc = _CACHE["nc"]

    b1p = np.zeros((7 * 128,), np.float32)
    b1p[:H1] = b1
    b1p = np.ascontiguousarray(b1p.reshape(7, 128).T)
    b2p = np.zeros((6 * 128,), np.float32)
    b2p[:H2] = b2
    b2p = np.ascontiguousarray(b2p.reshape(6, 128).T)
    b3p = np.full((1, OUTP), -1e30, np.float32)
    b3p[0, :OUT] = b3
    W3p = np.zeros((H2, OUTP), np.float32)
    W3p[:, :OUT] = W3

    # one vectorized pad+transpose pass for all shards
    xa = np.zeros((NCORES, TOK, DIM), np.float32)
    xa[:, :REAL_TOK] = x.reshape(NCORES, REAL_TOK, DIM)
    xTa = np.ascontiguousarray(xa.transpose(0, 2, 1))
    in_maps = []
    for i in range(NCORES):
        in_maps.append({
            "xT": xTa[i], "W1": W1, "W2": W2, "W3": W3p,
            "b1": b1p, "b2": b2p, "b3": b3p,
        })
    _CACHE["in_maps"] = in_maps

    results = None
    try:
        # restage every call: the staged device arrays must reflect THESE
        # inputs (a cached executor would replay the previous call's data)
        execute = _build_staged(nc, in_maps)
        _CACHE["execute"] = execute
        for attempt in range(3):
            try:
                results = execute()
                break
            except Exception:
                if attempt == 2:
                    raise
                import time as _time
                _time.sleep(5)
    except Exception:
        _CACHE.pop("execute", None)
        for attempt in range(3):
            try:
                res = run_bass_kernel_spmd(nc, in_maps,
                                           core_ids=list(range(NCORES)))
                results = res.results
                break
            except Exception:
                # transient NRT exec-unit wedge: cool down, then retry
                if attempt == 2:
                    raise
                import time as _time
                _time.sleep(10)

    parts, gparts = [], []
    for i in range(NCORES):
        codes = results[i]["codes"]          # [128, 32]
        parts.append(codes.T.reshape(-1)[:REAL_TOK])   # token t = s*128+p
        gparts.append(results[i]["gaps"].T.reshape(-1)[:REAL_TOK])
    code = np.concatenate(parts).astype(np.int32)   # [32000]
    gap = np.concatenate(gparts).astype(np.float32)

    # fp32r argmax can flip near-ties; recompute uncertain tokens exactly
    unc = np.flatnonzero(gap < 1e-2)
    if unc.size:
        xf = x.reshape(-1, DIM)[unc].astype(np.float32)
        h = xf @ W1 + b1
        h = np.where(h >= 0, h, np.float32(0.01) * h).astype(np.float32)
        h = h @ W2 + b2
        h = np.where(h >= 0, h, np.float32(0.01) * h).astype(np.float32)
        lg = h @ W3 + b3
        code[unc] = np.argmax(lg, axis=-1).astype(np.int32)

    code = _forward_fill_exact(code)
    return code.reshape(B, T)


# revision 9
# speedup vs baseline: 15198.9613x; 1.0646x over previous
"""Trainium2 Bass kernel for nn_LinearQuantizerModel.

MLP 1024->894->763->501 (leaky_relu 0.01) + argmax over classes + exact
forward-fill of stop tokens (==500) done on host.

Sharding: data-parallel over batch B=16 across 8 cores (2 batches/core),
weights replicated. Per core 4000 tokens padded to 4096 = 8 chunks x 512.

Device layout: features on partitions, tokens on free axis. x is
transposed on host so DMA loads are contiguous. Matmuls run in fp32r
(full-rate PE). Layer-3 flips orientation (stationary = H2T token tile,
moving = W3) so logits land [tokens, 501] in PSUM for vector argmax.

Execution: inputs are staged on the 8 devices once (device_put with a
"core"-sharded mesh) and the bass module is AOT-compiled once into a
fast-dispatch jax Compiled. Steady-state execution is then a single
async dispatch + one batched device_get of the two small outputs —
one tunnel roundtrip instead of re-uploading ~200 MB per call the way
per-call run_bass_kernel_spmd does. run_bass_kernel_spmd remains as a
fallback path.

build_kernel(nruns=N) additionally emits a benchmark variant whose whole
body (weight loads included) re-executes N times inside a tc.For_i loop;
one NEFF execution then runs the kernel N times back-to-back, letting
wall/N measure per-iteration device time with launch/tunnel overhead
amortized away (NTFF tracing is unavailable under axon).
"""

import contextlib

import numpy as np

import concourse.bass as bass
import concourse.mybir as mybir
import concourse.tile as tile
from concourse import bacc
from concourse.bass_utils import run_bass_kernel_spmd

B, T, DIM, H1, H2, OUT = 16, 2000, 1024, 894, 763, 501
OUTP = 512  # class dim padded for fp32r ISA (even/aligned free dim)
VOCAB = 500
MAX_ITERS = 10000
NCORES = 8
TOK = 4096          # padded tokens per core (4000 real)
REAL_TOK = 4000
NCHUNK = 8
CH = 512            # tokens per chunk
NSUB = TOK // 128   # 32 code columns

F32 = mybir.dt.float32
F32R = mybir.dt.float32r
MM_DT = F32R        # fp32r: full-rate PE, ~1e-4 matmul precision

_CACHE = {}


def _ceil(a, b):
    return (a + b - 1) // b


def build_kernel(nruns=1, bias_fold=True):
    """bias_fold: materialize b3 broadcast [128, OUTP] once per run with a
    single K=1 matmul, drop the per-subtile bias matmul (32/run), and fuse
    the bias add into the PSUM->SBUF logit move on the DVE (offloads the
    32 ACT copies too). Bitwise-equivalent argmax up to fp32 reassociation
    (~1e-7, far under the 1e-2 host-recompute threshold)."""
    nc = bacc.Bacc(target_bir_lowering=False)

    xT = nc.dram_tensor("xT", [DIM, TOK], MM_DT, kind="ExternalInput")
    W1d = nc.dram_tensor("W1", [DIM, H1], MM_DT, kind="ExternalInput")
    W2d = nc.dram_tensor("W2", [H1, H2], MM_DT, kind="ExternalInput")
    W3d = nc.dram_tensor("W3", [H2, OUTP], MM_DT, kind="ExternalInput")
    b1d = nc.dram_tensor("b1", [128, 7], F32, kind="ExternalInput")
    b2d = nc.dram_tensor("b2", [128, 6], F32, kind="ExternalInput")
    b3d = nc.dram_tensor("b3", [1, OUTP], MM_DT, kind="ExternalInput")
    codes_d = nc.dram_tensor("codes", [128, NSUB], mybir.dt.int32,
                             kind="ExternalOutput")
    gaps_d = nc.dram_tensor("gaps", [128, NSUB], F32, kind="ExternalOutput")

    KC1 = _ceil(DIM, 128)   # 8 (exact)
    KC2 = _ceil(H1, 128)    # 7, last 126
    KC3 = _ceil(H2, 128)    # 6, last 123
    MT1 = _ceil(H1, 128)    # 7, last 126
    MT2 = _ceil(H2, 128)    # 6, last 123

    LR = mybir.ActivationFunctionType.Lrelu

    with tile.TileContext(nc) as tc:
        with (
            tc.tile_pool(name="wpool", bufs=1) as wp,
            tc.tile_pool(name="xpool", bufs=3) as xp,
            tc.tile_pool(name="hpool", bufs=2) as hp,
            tc.tile_pool(name="spool", bufs=3) as sp,
            tc.tile_pool(name="cpool", bufs=1) as cp,
            tc.tile_pool(name="ps12", bufs=4, space="PSUM") as ps12,
            tc.tile_pool(name="ps3", bufs=3, space="PSUM") as ps3,
        ):
            loop = tc.For_i(0, nruns, 1) if nruns > 1 else (
                contextlib.nullcontext())
            with loop:
                # ---- weights / biases (loaded once per run) ----
                w1 = wp.tile([128, KC1, H1], MM_DT, tag="w1")
                nc.sync.dma_start(
                    out=w1, in_=W1d[:].rearrange("(kc p) m -> p kc m", p=128))
                w2 = wp.tile([128, KC2, H2], MM_DT, tag="w2")
                nc.sync.dma_start(
                    out=w2[:, 0:6, :],
                    in_=W2d[0:768, :].rearrange("(kc p) m -> p kc m", p=128))
                nc.sync.dma_start(
                    out=w2[0:126, 6, :], in_=W2d[768:894, :])
                w3 = wp.tile([128, KC3, OUTP], MM_DT, tag="w3")
                nc.sync.dma_start(
                    out=w3[:, 0:5, :],
                    in_=W3d[0:640, :].rearrange("(kc p) m -> p kc m", p=128))
                nc.sync.dma_start(
                    out=w3[0:123, 5, :], in_=W3d[640:763, :])
                b1 = wp.tile([128, 7], F32, tag="b1")
                nc.sync.dma_start(out=b1, in_=b1d[:])
                b2 = wp.tile([128, 6], F32, tag="b2")
                nc.sync.dma_start(out=b2, in_=b2d[:])
                b3 = wp.tile([1, OUTP], MM_DT, tag="b3")
                nc.sync.dma_start(out=b3, in_=b3d[:])
                ones_f = wp.tile([1, 128], F32, tag="ones_f")
                nc.vector.memset(ones_f, 1.0)
                ones = wp.tile([1, 128], MM_DT, tag="ones")
                nc.vector.tensor_copy(ones, ones_f)

                if bias_fold:
                    pb = ps3.tile([128, OUTP], F32, tag="plog")
                    nc.tensor.matmul(pb, ones, b3, start=True, stop=True)
                    b3b = wp.tile([128, OUTP], F32, tag="b3b")
                    nc.vector.tensor_copy(b3b, pb)

                codes_sb = cp.tile([128, NSUB], mybir.dt.int32, tag="codes")
                gaps_sb = cp.tile([128, NSUB], F32, tag="gaps")

                for c in range(NCHUNK):
                    xs = xp.tile([128, KC1, CH], MM_DT, tag="xslab")
                    nc.sync.dma_start(
                        out=xs,
                        in_=xT[:, c * CH:(c + 1) * CH].rearrange(
                            "(kc p) t -> p kc t", p=128))

                    # ---- layer 1: h1T[m*128+p, t] ----
                    h1t = hp.tile([128, KC2, CH], MM_DT, tag="h1t")
                    for mt in range(MT1):
                        m0 = mt * 128
                        mw = min(128, H1 - m0)
                        pt = ps12.tile([128, CH], F32, tag="pmm")
                        for kc in range(KC1):
                            nc.tensor.matmul(
                                pt[:mw, :], w1[:, kc, m0:m0 + mw],
                                xs[:, kc, :],
                                start=(kc == 0), stop=(kc == KC1 - 1))
                        nc.scalar.activation(
                            h1t[:mw, mt, :], pt[:mw, :], LR,
                            bias=b1[:mw, mt:mt + 1], scale=1.0, alpha=0.01)

                    # ---- layer 2 ----
                    h2t = hp.tile([128, KC3, CH], MM_DT, tag="h2t")
                    for mt in range(MT2):
                        m0 = mt * 128
                        mw = min(128, H2 - m0)
                        pt = ps12.tile([128, CH], F32, tag="pmm")
                        for kc in range(KC2):
                            kw = min(128, H1 - kc * 128)
                            nc.tensor.matmul(
                                pt[:mw, :], w2[:kw, kc, m0:m0 + mw],
                                h1t[:kw, kc, :],
                                start=(kc == 0), stop=(kc == KC2 - 1))
                        nc.scalar.activation(
                            h2t[:mw, mt, :], pt[:mw, :], LR,
                            bias=b2[:mw, mt:mt + 1], scale=1.0, alpha=0.01)

                    # ---- layer 3 + argmax: per 128-token subtile ----
                    for s in range(4):
                        t0 = s * 128
                        pl = ps3.tile([128, OUTP], F32, tag="plog")
                        if not bias_fold:
                            nc.tensor.matmul(pl, ones, b3, start=True,
                                             stop=False)
                        for kc in range(KC3):
                            kw = min(128, H2 - kc * 128)
                            nc.tensor.matmul(
                                pl, h2t[:kw, kc, t0:t0 + 128], w3[:kw, kc, :],
                                start=(bias_fold and kc == 0),
                                stop=(kc == KC3 - 1))
                        logit = sp.tile([128, OUTP], F32, tag="logit")
                        if bias_fold:
                            nc.vector.tensor_add(logit, pl, b3b)
                        else:
                            nc.scalar.copy(logit, pl)
                        mx8 = sp.tile([128, 8], F32, tag="mx8")
                        ix8 = sp.tile([128, 8], mybir.dt.uint32, tag="ix8")
                        nc.vector.max(mx8, logit)
                        nc.vector.max_index(ix8, mx8, logit)
                        col = c * 4 + s
                        nc.vector.tensor_copy(
                            codes_sb.bitcast(mybir.dt.uint32)[:, col:col + 1],
                            ix8[:, 0:1])
                        nc.vector.tensor_sub(
                            gaps_sb[:, col:col + 1], mx8[:, 0:1], mx8[:, 1:2])

                nc.sync.dma_start(out=codes_d[:], in_=codes_sb)
                nc.sync.dma_start(out=gaps_d[:], in_=gaps_sb)

    nc.finalize()
    return nc


# ---------------------------------------------------------------------------
# Staged executor: device-resident inputs + AOT-compiled fast-dispatch call.
# ---------------------------------------------------------------------------

def _introspect(nc):
    import jax
    partition_name = (nc.partition_id_tensor.name
                      if nc.partition_id_tensor else None)
    in_names, out_names, out_avals = [], [], []
    for alloc in nc.m.functions[0].allocations:
        if not isinstance(alloc, mybir.MemoryLocationSet):
            continue
        name = alloc.memorylocations[0].name
        if alloc.kind == "ExternalInput":
            if name != partition_name:
                in_names.append(name)
        elif alloc.kind == "ExternalOutput":
            out_names.append(name)
            out_avals.append(jax.core.ShapedArray(
                tuple(alloc.tensor_shape), mybir.dt.np(alloc.dtype)))
    return partition_name, in_names, out_names, out_avals


def _aot_compile(nc, dev_in, dev_zero):
    """AOT-compile nc into a reusable (fast-dispatch) jax Compiled taking
    (dev_in..., dev_zero...) sharded over the 8-core mesh."""
    import jax
    from jax.sharding import Mesh, PartitionSpec
    from jax.experimental.shard_map import shard_map
    from concourse.bass2jax import (_bass_exec_p, install_neuronx_cc_hook,
                                    fast_dispatch_compile,
                                    partition_id_tensor)

    install_neuronx_cc_hook()
    partition_name, in_names, out_names, out_avals = _introspect(nc)
    n_params = len(in_names)
    all_names = list(in_names) + list(out_names)
    if partition_name is not None:
        all_names.append(partition_name)

    def _body(*args):
        operands = list(args)
        if partition_name is not None:
            operands.append(partition_id_tensor())
        outs = _bass_exec_p.bind(
            *operands,
            out_avals=tuple(out_avals),
            in_names=tuple(all_names),
            out_names=tuple(out_names),
            lowering_input_output_aliases=(),
            sim_require_finite=True,
            sim_require_nnan=True,
            nc=nc,
        )
        return tuple(outs)

    devices = jax.devices()[:NCORES]
    mesh = Mesh(np.asarray(devices), ("core",))
    in_specs = (PartitionSpec("core"),) * (n_params + len(out_names))
    out_specs = (PartitionSpec("core"),) * len(out_names)
    fn = shard_map(_body, mesh=mesh, in_specs=in_specs, out_specs=out_specs,
                   check_rep=False)

    def _compile_plain():
        return jax.jit(fn, keep_unused=True).lower(
            *dev_in, *dev_zero).compile()

    try:
        return fast_dispatch_compile(_compile_plain)
    except Exception:
        return _compile_plain()


def _stage(nc, in_maps):
    """device_put the per-core inputs (concat over the core axis) + zeroed
    output buffers onto the 8-device mesh."""
    import jax
    from jax.sharding import Mesh, PartitionSpec, NamedSharding

    partition_name, in_names, out_names, out_avals = _introspect(nc)
    devices = jax.devices()[:NCORES]
    mesh = Mesh(np.asarray(devices), ("core",))
    sharding = NamedSharding(mesh, PartitionSpec("core"))

    concat_in = [np.concatenate([np.asarray(m[n]) for m in in_maps], axis=0)
                 for n in in_names]
    concat_zero = [np.zeros((NCORES * a.shape[0], *a.shape[1:]), a.dtype)
                   for a in out_avals]
    dev_in = [jax.device_put(a, sharding) for a in concat_in]
    dev_zero = [jax.device_put(a, sharding) for a in concat_zero]
    for a in dev_in + dev_zero:
        a.block_until_ready()
    return dev_in, dev_zero


def _build_staged(nc, in_maps):
    """Stage inputs + AOT-compile; returns execute() -> per-core result
    dicts at a cost of one async dispatch + one batched device fetch."""
    import jax

    _, _, out_names, out_avals = _introspect(nc)
    dev_in, dev_zero = _stage(nc, in_maps)
    compiled = _aot_compile(nc, dev_in, dev_zero)

    _CACHE["compiled"] = compiled
    _CACHE["dev_in"] = dev_in
    _CACHE["dev_zero"] = dev_zero

    def execute():
        outs = compiled(*dev_in, *dev_zero)
        host = jax.device_get(tuple(outs))  # one batched roundtrip
        return [
            {name: np.asarray(host[i]).reshape(NCORES, *out_avals[i].shape)[c]
             for i, name in enumerate(out_names)}
            for c in range(NCORES)
        ]

    return execute


def _forward_fill_exact(code_flat: np.ndarray) -> np.ndarray:
    """Exact equivalent of the reference jax while-loop fill."""
    n = code_flat.shape[0]
    mask = code_flat == VOCAB
    if not mask.any():
        return code_flat
    if mask.all():
        return code_flat
    idx = np.where(~mask, np.arange(n), -1)
    fill = np.maximum.accumulate(idx)
    # wrap-around: positions before first non-stop take the last non-stop
    last = np.max(idx)
    dist = np.arange(n) - fill
    wrapped = fill < 0
    fill = np.where(wrapped, last, fill)
    dist = np.where(wrapped, np.arange(n) + (n - last), dist)
    out = code_flat[fill]
    # faithful MAX_ITERS cap: stops further than MAX_ITERS remain
    out = np.where(mask & (dist > MAX_ITERS), VOCAB, out)
    out = np.where(mask, out, code_flat)
    return out.astype(np.int32)


def kernel(x, W1, b1, W2, b2, W3, b3):
    x = np.asarray(x, dtype=np.float32)
    W1 = np.ascontiguousarray(np.asarray(W1, dtype=np.float32))
    W2 = np.ascontiguousarray(np.asarray(W2, dtype=np.float32))
    W3 = np.ascontiguousarray(np.asarray(W3, dtype=np.float32))
    b1 = np.asarray(b1, dtype=np.float32)
    b2 = np.asarray(b2, dtype=np.float32)
    b3 = np.asarray(b3, dtype=np.float32)

    if "nc" not in _CACHE:
        _CACHE["nc"] = build_kernel()
    nc = _CACHE["nc"]

    b1p = np.zeros((7 * 128,), np.float32)
    b1p[:H1] = b1
    b1p = np.ascontiguousarray(b1p.reshape(7, 128).T)
    b2p = np.zeros((6 * 128,), np.float32)
    b2p[:H2] = b2
    b2p = np.ascontiguousarray(b2p.reshape(6, 128).T)
    b3p = np.full((1, OUTP), -1e30, np.float32)
    b3p[0, :OUT] = b3
    W3p = np.zeros((H2, OUTP), np.float32)
    W3p[:, :OUT] = W3

    # one vectorized pad+transpose pass for all shards
    xa = np.zeros((NCORES, TOK, DIM), np.float32)
    xa[:, :REAL_TOK] = x.reshape(NCORES, REAL_TOK, DIM)
    xTa = np.ascontiguousarray(xa.transpose(0, 2, 1))
    in_maps = []
    for i in range(NCORES):
        in_maps.append({
            "xT": xTa[i], "W1": W1, "W2": W2, "W3": W3p,
            "b1": b1p, "b2": b2p, "b3": b3p,
        })
    _CACHE["in_maps"] = in_maps

    results = None
    try:
        # restage every call: the staged device arrays must reflect THESE
        # inputs (a cached executor would replay the previous call's data)
        execute = _build_staged(nc, in_maps)
        _CACHE["execute"] = execute
        for attempt in range(3):
            try:
                results = execute()
                break
            except Exception:
                if attempt == 2:
                    raise
                import time as _time
                _time.sleep(5)
    except Exception:
        _CACHE.pop("execute", None)
        for attempt in range(3):
            try:
                res = run_bass_kernel_spmd(nc, in_maps,
                                           core_ids=list(range(NCORES)))
                results = res.results
                break
            except Exception:
                # transient NRT exec-unit wedge: cool down, then retry
                if attempt == 2:
                    raise
                import time as _time
                _time.sleep(10)

    parts, gparts = [], []
    for i in range(NCORES):
        codes = results[i]["codes"]          # [128, 32]
        parts.append(codes.T.reshape(-1)[:REAL_TOK])   # token t = s*128+p
        gparts.append(results[i]["gaps"].T.reshape(-1)[:REAL_TOK])
    code = np.concatenate(parts).astype(np.int32)   # [32000]
    gap = np.concatenate(gparts).astype(np.float32)

    # fp32r argmax can flip near-ties; recompute uncertain tokens exactly
    unc = np.flatnonzero(gap < 1e-2)
    if unc.size:
        xf = x.reshape(-1, DIM)[unc].astype(np.float32)
        h = xf @ W1 + b1
        h = np.where(h >= 0, h, np.float32(0.01) * h).astype(np.float32)
        h = h @ W2 + b2
        h = np.where(h >= 0, h, np.float32(0.01) * h).astype(np.float32)
        lg = h @ W3 + b3
        code[unc] = np.argmax(lg, axis=-1).astype(np.int32)

    code = _forward_fill_exact(code)
    return code.reshape(B, T)


# revision 12
# speedup vs baseline: 15587.2927x; 1.0255x over previous
"""Trainium2 Bass kernel for nn_LinearQuantizerModel.

MLP 1024->894->763->501 (leaky_relu 0.01) + argmax over classes + exact
forward-fill of stop tokens (==500) done on host.

Sharding: data-parallel over batch B=16 across 8 cores (2 batches/core),
weights replicated. Per core 4000 tokens padded to 4096 = 8 chunks x 512.

Device layout: features on partitions, tokens on free axis. x is
transposed on host so DMA loads are contiguous. Matmuls run in fp32r
(full-rate PE). Layer-3 flips orientation (stationary = H2T token tile,
moving = W3) so logits land [tokens, 501] in PSUM for vector argmax.

Execution: inputs are staged on the 8 devices once (device_put with a
"core"-sharded mesh) and the bass module is AOT-compiled once into a
fast-dispatch jax Compiled. Steady-state execution is then a single
async dispatch + one batched device_get of the two small outputs —
one tunnel roundtrip instead of re-uploading ~200 MB per call the way
per-call run_bass_kernel_spmd does. run_bass_kernel_spmd remains as a
fallback path.

build_kernel(nruns=N) additionally emits a benchmark variant whose whole
body (weight loads included) re-executes N times inside a tc.For_i loop;
one NEFF execution then runs the kernel N times back-to-back, letting
wall/N measure per-iteration device time with launch/tunnel overhead
amortized away (NTFF tracing is unavailable under axon).
"""

import contextlib

import numpy as np

import concourse.bass as bass
import concourse.mybir as mybir
import concourse.tile as tile
from concourse import bacc
from concourse.bass_utils import run_bass_kernel_spmd

B, T, DIM, H1, H2, OUT = 16, 2000, 1024, 894, 763, 501
OUTP = 512  # class dim padded for fp32r ISA (even/aligned free dim)
VOCAB = 500
MAX_ITERS = 10000
NCORES = 8
TOK = 4096          # padded tokens per core (4000 real)
REAL_TOK = 4000
NCHUNK = 8
CH = 512            # tokens per chunk
NSUB = TOK // 128   # 32 code columns

F32 = mybir.dt.float32
F32R = mybir.dt.float32r
MM_DT = F32R        # fp32r: full-rate PE, ~1e-4 matmul precision

_CACHE = {}


def _ceil(a, b):
    return (a + b - 1) // b


def build_kernel(nruns=1, bias_fold=True, ps3_bufs=3, x2=False, xbufs=3):
    """bias_fold: materialize b3 broadcast [128, OUTP] once per run with a
    single K=1 matmul, drop the per-subtile bias matmul (32/run), and fuse
    the bias add into the PSUM->SBUF logit move on the DVE (offloads the
    32 ACT copies too). Bitwise-equivalent argmax up to fp32 reassociation
    (~1e-7, far under the 1e-2 host-recompute threshold).
    ps3_bufs: PSUM banks for the L3 logit accumulators (4 lets all four
    128-token subtiles accumulate concurrently; ps12 4 + ps3 4 = all 8).
    x2: load x in 4 x 4MB double-chunk DMAs instead of 8 x 2MB (halves
    per-transfer fixed cost; needs xbufs<=2 for SBUF budget)."""
    nc = bacc.Bacc(target_bir_lowering=False)

    xT = nc.dram_tensor("xT", [DIM, TOK], MM_DT, kind="ExternalInput")
    W1d = nc.dram_tensor("W1", [DIM, H1], MM_DT, kind="ExternalInput")
    W2d = nc.dram_tensor("W2", [H1, H2], MM_DT, kind="ExternalInput")
    W3d = nc.dram_tensor("W3", [H2, OUTP], MM_DT, kind="ExternalInput")
    b1d = nc.dram_tensor("b1", [128, 7], F32, kind="ExternalInput")
    b2d = nc.dram_tensor("b2", [128, 6], F32, kind="ExternalInput")
    b3d = nc.dram_tensor("b3", [1, OUTP], MM_DT, kind="ExternalInput")
    codes_d = nc.dram_tensor("codes", [128, NSUB], mybir.dt.int32,
                             kind="ExternalOutput")
    gaps_d = nc.dram_tensor("gaps", [128, NSUB], F32, kind="ExternalOutput")

    KC1 = _ceil(DIM, 128)   # 8 (exact)
    KC2 = _ceil(H1, 128)    # 7, last 126
    KC3 = _ceil(H2, 128)    # 6, last 123
    MT1 = _ceil(H1, 128)    # 7, last 126
    MT2 = _ceil(H2, 128)    # 6, last 123

    LR = mybir.ActivationFunctionType.Lrelu

    with tile.TileContext(nc) as tc:
        with (
            tc.tile_pool(name="wpool", bufs=1) as wp,
            tc.tile_pool(name="xpool", bufs=xbufs) as xp,
            tc.tile_pool(name="hpool", bufs=2) as hp,
            tc.tile_pool(name="spool", bufs=3) as sp,
            tc.tile_pool(name="cpool", bufs=1) as cp,
            tc.tile_pool(name="ps12", bufs=4, space="PSUM") as ps12,
            tc.tile_pool(name="ps3", bufs=ps3_bufs, space="PSUM") as ps3,
        ):
            loop = tc.For_i(0, nruns, 1) if nruns > 1 else (
                contextlib.nullcontext())
            with loop:
                # ---- weights / biases (loaded once per run) ----
                w1 = wp.tile([128, KC1, H1], MM_DT, tag="w1")
                nc.sync.dma_start(
                    out=w1, in_=W1d[:].rearrange("(kc p) m -> p kc m", p=128))
                w2 = wp.tile([128, KC2, H2], MM_DT, tag="w2")
                nc.sync.dma_start(
                    out=w2[:, 0:6, :],
                    in_=W2d[0:768, :].rearrange("(kc p) m -> p kc m", p=128))
                nc.sync.dma_start(
                    out=w2[0:126, 6, :], in_=W2d[768:894, :])
                w3 = wp.tile([128, KC3, OUTP], MM_DT, tag="w3")
                nc.sync.dma_start(
                    out=w3[:, 0:5, :],
                    in_=W3d[0:640, :].rearrange("(kc p) m -> p kc m", p=128))
                nc.sync.dma_start(
                    out=w3[0:123, 5, :], in_=W3d[640:763, :])
                b1 = wp.tile([128, 7], F32, tag="b1")
                nc.sync.dma_start(out=b1, in_=b1d[:])
                b2 = wp.tile([128, 6], F32, tag="b2")
                nc.sync.dma_start(out=b2, in_=b2d[:])
                b3 = wp.tile([1, OUTP], MM_DT, tag="b3")
                nc.sync.dma_start(out=b3, in_=b3d[:])
                ones_f = wp.tile([1, 128], F32, tag="ones_f")
                nc.vector.memset(ones_f, 1.0)
                ones = wp.tile([1, 128], MM_DT, tag="ones")
                nc.vector.tensor_copy(ones, ones_f)

                if bias_fold:
                    pb = ps3.tile([128, OUTP], F32, tag="plog")
                    nc.tensor.matmul(pb, ones, b3, start=True, stop=True)
                    b3b = wp.tile([128, OUTP], F32, tag="b3b")
                    nc.vector.tensor_copy(b3b, pb)

                codes_sb = cp.tile([128, NSUB], mybir.dt.int32, tag="codes")
                gaps_sb = cp.tile([128, NSUB], F32, tag="gaps")

                xs2 = None
                for c in range(NCHUNK):
                    if x2:
                        if c % 2 == 0:
                            xs2 = xp.tile([128, KC1, 2 * CH], MM_DT,
                                          tag="xslab")
                            nc.sync.dma_start(
                                out=xs2,
                                in_=xT[:, c * CH:(c + 2) * CH].rearrange(
                                    "(kc p) t -> p kc t", p=128))
                        off = (c % 2) * CH
                        xs = xs2[:, :, off:off + CH]
                    else:
                        xs = xp.tile([128, KC1, CH], MM_DT, tag="xslab")
                        nc.sync.dma_start(
                            out=xs,
                            in_=xT[:, c * CH:(c + 1) * CH].rearrange(
                                "(kc p) t -> p kc t", p=128))

                    # ---- layer 1: h1T[m*128+p, t] ----
                    h1t = hp.tile([128, KC2, CH], MM_DT, tag="h1t")
                    for mt in range(MT1):
                        m0 = mt * 128
                        mw = min(128, H1 - m0)
                        pt = ps12.tile([128, CH], F32, tag="pmm")
                        for kc in range(KC1):
                            nc.tensor.matmul(
                                pt[:mw, :], w1[:, kc, m0:m0 + mw],
                                xs[:, kc, :],
                                start=(kc == 0), stop=(kc == KC1 - 1))
                        nc.scalar.activation(
                            h1t[:mw, mt, :], pt[:mw, :], LR,
                            bias=b1[:mw, mt:mt + 1], scale=1.0, alpha=0.01)

                    # ---- layer 2 ----
                    h2t = hp.tile([128, KC3, CH], MM_DT, tag="h2t")
                    for mt in range(MT2):
                        m0 = mt * 128
                        mw = min(128, H2 - m0)
                        pt = ps12.tile([128, CH], F32, tag="pmm")
                        for kc in range(KC2):
                            kw = min(128, H1 - kc * 128)
                            nc.tensor.matmul(
                                pt[:mw, :], w2[:kw, kc, m0:m0 + mw],
                                h1t[:kw, kc, :],
                                start=(kc == 0), stop=(kc == KC2 - 1))
                        nc.scalar.activation(
                            h2t[:mw, mt, :], pt[:mw, :], LR,
                            bias=b2[:mw, mt:mt + 1], scale=1.0, alpha=0.01)

                    # ---- layer 3 + argmax: per 128-token subtile ----
                    for s in range(4):
                        t0 = s * 128
                        pl = ps3.tile([128, OUTP], F32, tag="plog")
                        if not bias_fold:
                            nc.tensor.matmul(pl, ones, b3, start=True,
                                             stop=False)
                        for kc in range(KC3):
                            kw = min(128, H2 - kc * 128)
                            nc.tensor.matmul(
                                pl, h2t[:kw, kc, t0:t0 + 128], w3[:kw, kc, :],
                                start=(bias_fold and kc == 0),
                                stop=(kc == KC3 - 1))
                        logit = sp.tile([128, OUTP], F32, tag="logit")
                        if bias_fold:
                            nc.vector.tensor_add(logit, pl, b3b)
                        else:
                            nc.scalar.copy(logit, pl)
                        mx8 = sp.tile([128, 8], F32, tag="mx8")
                        ix8 = sp.tile([128, 8], mybir.dt.uint32, tag="ix8")
                        nc.vector.max(mx8, logit)
                        nc.vector.max_index(ix8, mx8, logit)
                        col = c * 4 + s
                        nc.vector.tensor_copy(
                            codes_sb.bitcast(mybir.dt.uint32)[:, col:col + 1],
                            ix8[:, 0:1])
                        nc.vector.tensor_sub(
                            gaps_sb[:, col:col + 1], mx8[:, 0:1], mx8[:, 1:2])

                nc.sync.dma_start(out=codes_d[:], in_=codes_sb)
                nc.sync.dma_start(out=gaps_d[:], in_=gaps_sb)

    nc.finalize()
    return nc


# ---------------------------------------------------------------------------
# Staged executor: device-resident inputs + AOT-compiled fast-dispatch call.
# ---------------------------------------------------------------------------

def _introspect(nc):
    import jax
    partition_name = (nc.partition_id_tensor.name
                      if nc.partition_id_tensor else None)
    in_names, out_names, out_avals = [], [], []
    for alloc in nc.m.functions[0].allocations:
        if not isinstance(alloc, mybir.MemoryLocationSet):
            continue
        name = alloc.memorylocations[0].name
        if alloc.kind == "ExternalInput":
            if name != partition_name:
                in_names.append(name)
        elif alloc.kind == "ExternalOutput":
            out_names.append(name)
            out_avals.append(jax.core.ShapedArray(
                tuple(alloc.tensor_shape), mybir.dt.np(alloc.dtype)))
    return partition_name, in_names, out_names, out_avals


def _aot_compile(nc, dev_in, dev_zero):
    """AOT-compile nc into a reusable (fast-dispatch) jax Compiled taking
    (dev_in..., dev_zero...) sharded over the 8-core mesh."""
    import jax
    from jax.sharding import Mesh, PartitionSpec
    from jax.experimental.shard_map import shard_map
    from concourse.bass2jax import (_bass_exec_p, install_neuronx_cc_hook,
                                    fast_dispatch_compile,
                                    partition_id_tensor)

    install_neuronx_cc_hook()
    partition_name, in_names, out_names, out_avals = _introspect(nc)
    n_params = len(in_names)
    all_names = list(in_names) + list(out_names)
    if partition_name is not None:
        all_names.append(partition_name)

    def _body(*args):
        operands = list(args)
        if partition_name is not None:
            operands.append(partition_id_tensor())
        outs = _bass_exec_p.bind(
            *operands,
            out_avals=tuple(out_avals),
            in_names=tuple(all_names),
            out_names=tuple(out_names),
            lowering_input_output_aliases=(),
            sim_require_finite=True,
            sim_require_nnan=True,
            nc=nc,
        )
        return tuple(outs)

    devices = jax.devices()[:NCORES]
    mesh = Mesh(np.asarray(devices), ("core",))
    in_specs = (PartitionSpec("core"),) * (n_params + len(out_names))
    out_specs = (PartitionSpec("core"),) * len(out_names)
    fn = shard_map(_body, mesh=mesh, in_specs=in_specs, out_specs=out_specs,
                   check_rep=False)

    def _compile_plain():
        return jax.jit(fn, keep_unused=True).lower(
            *dev_in, *dev_zero).compile()

    try:
        return fast_dispatch_compile(_compile_plain)
    except Exception:
        return _compile_plain()


def _stage(nc, in_maps):
    """device_put the per-core inputs (concat over the core axis) + zeroed
    output buffers onto the 8-device mesh."""
    import jax
    from jax.sharding import Mesh, PartitionSpec, NamedSharding

    partition_name, in_names, out_names, out_avals = _introspect(nc)
    devices = jax.devices()[:NCORES]
    mesh = Mesh(np.asarray(devices), ("core",))
    sharding = NamedSharding(mesh, PartitionSpec("core"))

    concat_in = [np.concatenate([np.asarray(m[n]) for m in in_maps], axis=0)
                 for n in in_names]
    concat_zero = [np.zeros((NCORES * a.shape[0], *a.shape[1:]), a.dtype)
                   for a in out_avals]
    dev_in = [jax.device_put(a, sharding) for a in concat_in]
    dev_zero = [jax.device_put(a, sharding) for a in concat_zero]
    for a in dev_in + dev_zero:
        a.block_until_ready()
    return dev_in, dev_zero


def _build_staged(nc, in_maps):
    """Stage inputs + AOT-compile; returns execute() -> per-core result
    dicts at a cost of one async dispatch + one batched device fetch."""
    import jax

    _, _, out_names, out_avals = _introspect(nc)
    dev_in, dev_zero = _stage(nc, in_maps)
    compiled = _aot_compile(nc, dev_in, dev_zero)

    _CACHE["compiled"] = compiled
    _CACHE["dev_in"] = dev_in
    _CACHE["dev_zero"] = dev_zero

    def execute():
        outs = compiled(*dev_in, *dev_zero)
        host = jax.device_get(tuple(outs))  # one batched roundtrip
        return [
            {name: np.asarray(host[i]).reshape(NCORES, *out_avals[i].shape)[c]
             for i, name in enumerate(out_names)}
            for c in range(NCORES)
        ]

    return execute


def _forward_fill_exact(code_flat: np.ndarray) -> np.ndarray:
    """Exact equivalent of the reference jax while-loop fill."""
    n = code_flat.shape[0]
    mask = code_flat == VOCAB
    if not mask.any():
        return code_flat
    if mask.all():
        return code_flat
    idx = np.where(~mask, np.arange(n), -1)
    fill = np.maximum.accumulate(idx)
    # wrap-around: positions before first non-stop take the last non-stop
    last = np.max(idx)
    dist = np.arange(n) - fill
    wrapped = fill < 0
    fill = np.where(wrapped, last, fill)
    dist = np.where(wrapped, np.arange(n) + (n - last), dist)
    out = code_flat[fill]
    # faithful MAX_ITERS cap: stops further than MAX_ITERS remain
    out = np.where(mask & (dist > MAX_ITERS), VOCAB, out)
    out = np.where(mask, out, code_flat)
    return out.astype(np.int32)


def kernel(x, W1, b1, W2, b2, W3, b3):
    x = np.asarray(x, dtype=np.float32)
    W1 = np.ascontiguousarray(np.asarray(W1, dtype=np.float32))
    W2 = np.ascontiguousarray(np.asarray(W2, dtype=np.float32))
    W3 = np.ascontiguousarray(np.asarray(W3, dtype=np.float32))
    b1 = np.asarray(b1, dtype=np.float32)
    b2 = np.asarray(b2, dtype=np.float32)
    b3 = np.asarray(b3, dtype=np.float32)

    if "nc" not in _CACHE:
        _CACHE["nc"] = build_kernel()
    nc = _CACHE["nc"]

    b1p = np.zeros((7 * 128,), np.float32)
    b1p[:H1] = b1
    b1p = np.ascontiguousarray(b1p.reshape(7, 128).T)
    b2p = np.zeros((6 * 128,), np.float32)
    b2p[:H2] = b2
    b2p = np.ascontiguousarray(b2p.reshape(6, 128).T)
    b3p = np.full((1, OUTP), -1e30, np.float32)
    b3p[0, :OUT] = b3
    W3p = np.zeros((H2, OUTP), np.float32)
    W3p[:, :OUT] = W3

    # one vectorized pad+transpose pass for all shards
    xa = np.zeros((NCORES, TOK, DIM), np.float32)
    xa[:, :REAL_TOK] = x.reshape(NCORES, REAL_TOK, DIM)
    xTa = np.ascontiguousarray(xa.transpose(0, 2, 1))
    in_maps = []
    for i in range(NCORES):
        in_maps.append({
            "xT": xTa[i], "W1": W1, "W2": W2, "W3": W3p,
            "b1": b1p, "b2": b2p, "b3": b3p,
        })
    _CACHE["in_maps"] = in_maps

    results = None
    try:
        # restage every call: the staged device arrays must reflect THESE
        # inputs (a cached executor would replay the previous call's data)
        execute = _build_staged(nc, in_maps)
        _CACHE["execute"] = execute
        for attempt in range(3):
            try:
                results = execute()
                break
            except Exception:
                if attempt == 2:
                    raise
                import time as _time
                _time.sleep(5)
    except Exception:
        _CACHE.pop("execute", None)
        for attempt in range(3):
            try:
                res = run_bass_kernel_spmd(nc, in_maps,
                                           core_ids=list(range(NCORES)))
                results = res.results
                break
            except Exception:
                # transient NRT exec-unit wedge: cool down, then retry
                if attempt == 2:
                    raise
                import time as _time
                _time.sleep(10)

    parts, gparts = [], []
    for i in range(NCORES):
        codes = results[i]["codes"]          # [128, 32]
        parts.append(codes.T.reshape(-1)[:REAL_TOK])   # token t = s*128+p
        gparts.append(results[i]["gaps"].T.reshape(-1)[:REAL_TOK])
    code = np.concatenate(parts).astype(np.int32)   # [32000]
    gap = np.concatenate(gparts).astype(np.float32)

    # fp32r argmax can flip near-ties; recompute uncertain tokens exactly
    unc = np.flatnonzero(gap < 1e-2)
    if unc.size:
        xf = x.reshape(-1, DIM)[unc].astype(np.float32)
        h = xf @ W1 + b1
        h = np.where(h >= 0, h, np.float32(0.01) * h).astype(np.float32)
        h = h @ W2 + b2
        h = np.where(h >= 0, h, np.float32(0.01) * h).astype(np.float32)
        lg = h @ W3 + b3
        code[unc] = np.argmax(lg, axis=-1).astype(np.int32)

    code = _forward_fill_exact(code)
    return code.reshape(B, T)
